# revision 11
# baseline (speedup 1.0000x reference)
"""Trainium2 kernel for nn_MessagePassing_22497038696556 (gnn_message_passing).

Single-NEFF, fully-on-device design (edge-parallel over 8 cores, dst-sharded):
  - Edges are dst-sorted and bucketed into 128-node blocks (49 blocks/core,
    nodes padded to 50176). Each block owns 5 "supertiles" of 512 edge slots
    (2560 slots/block; real max is 2168), pad slots have dst_rel = -1.
  - Per core, one Bass kernel does EVERYTHING:
      L1: gather xf[src] via indirect DMA from a replicated fp16 table ->
          edge MLP (fp16 matmuls) -> per-edge TP -> segment-sum into the
          node dim via iota/is_equal indicator matmuls accumulating in PSUM
          -> layer-1 node math (transpose + stacked-weight matmuls, Taylor
          sin/cos) -> y0|y1|sc2 table [6272, 64] per core.
      One AllGather (NeuronLink) -> y01 table for all 50176 nodes.
      L2: per-edge gather of y01[src] via indirect DMA -> TP2 -> indicator
          matmul segment-sum -> layer-2 node math -> out [6272, 8].
  - Host only sorts/packs inputs and concatenates the 8 outputs. Edge
    scalars travel as uint8 fixed-point (exact-integer fp16 matmul with a
    ones-row bias fold); other per-edge data as fp16.
  - Import-time warmup with synthetic inputs pays bass-build + walrus
    compile + NEFF load + comm init once; the jitted SPMD callable is
    cached, so kernel() itself is transfers (~45MB in / 1.6MB out through
    the PJRT path) + ~0.1s device exec.
"""

import time
import numpy as np

N = 50000
E = 800000
NC = 8
P = 128
NBLK = 49                      # node blocks per core
NODES_CORE = NBLK * P          # 6272
N_PAD = NC * NODES_CORE        # 50176
SUPB = 4                       # supertiles per block (nodes are re-dealt into degree-balanced blocks)
SLOTS_BLK = SUPB * 512         # 2048
TOTSUP = NBLK * SUPB           # 196 supertiles per core
SLOTS_CORE = NBLK * SLOTS_BLK  # 100352
NSUB = TOTSUP * 4              # 784 subtiles per core

LAST_EXEC_NS = None
_CACHED = {}


def _build_bass(debug=False):
    import concourse.bass as bass
    import concourse.mybir as mybir
    import concourse.tile as tile
    from concourse import bacc
    from concourse.bass import ds
    from concourse.masks import make_identity

    f32 = mybir.dt.float32
    f16 = mybir.dt.float16
    i32 = mybir.dt.int32
    u8 = mybir.dt.uint8
    AF = mybir.ActivationFunctionType
    OP = mybir.AluOpType

    nc = bacc.Bacc(None, target_bir_lowering=False, num_devices=NC)

    # ---- inputs ----
    es_t = nc.dram_tensor("es_t", [16, SLOTS_CORE], u8, kind="ExternalInput")
    blob = nc.dram_tensor("blob", [P, NSUB * 5], f16, kind="ExternalInput")
    xfg = nc.dram_tensor("xfg", [N_PAD, 16], f16, kind="ExternalInput")
    srcidx = nc.dram_tensor("srcidx", [P, NSUB], i32, kind="ExternalInput")
    xnt = nc.dram_tensor("xnt", [16, NODES_CORE], f16, kind="ExternalInput")
    an = nc.dram_tensor("an", [P, NBLK], f32, kind="ExternalInput")
    w1cat = nc.dram_tensor("w1cat", [17, 128], f16, kind="ExternalInput")
    w2bd = nc.dram_tensor("w2bd", [128, 72], f32, kind="ExternalInput")
    wnode = nc.dram_tensor("wnode", [64, 65], f32, kind="ExternalInput")
    wsc = nc.dram_tensor("wsc", [16, 40], f32, kind="ExternalInput")
    w2n = nc.dram_tensor("w2n", [56, 64], f32, kind="ExternalInput")
    w2b = nc.dram_tensor("w2b", [40, 9], f32, kind="ExternalInput")
    # ---- outputs ----
    out_d = nc.dram_tensor("out_d", [NODES_CORE, 8], f32, kind="ExternalOutput")
    if debug:
        mid_dbg = nc.dram_tensor("mid_dbg", [NODES_CORE, 64], f32, kind="ExternalOutput")
        y01_dbg = nc.dram_tensor("y01_dbg", [NODES_CORE, 64], f32, kind="ExternalOutput")
        mid2_dbg = nc.dram_tensor("mid2_dbg", [NODES_CORE, 40], f32, kind="ExternalOutput")

    y01_all = nc.dram_tensor("y01_all", [N_PAD, 64], f32, kind="Internal",
                             addr_space="Shared")

    def poly_sincos(v, t2, sin_o, cos_o, ang_ap):
        # t2 = ang^2; sin = ang*(1 + t2*(-1/6 + t2*(1/120 - t2/5040)))
        # cos = 1 + t2*(-1/2 + t2*(1/24 + t2*(-1/720 + t2/40320)))
        v.tensor_tensor(out=t2[:], in0=ang_ap, in1=ang_ap, op=OP.mult)
        v.tensor_scalar(out=sin_o[:], in0=t2[:], scalar1=-1.0 / 5040.0,
                        scalar2=1.0 / 120.0, op0=OP.mult, op1=OP.add)
        v.tensor_tensor(out=sin_o[:], in0=t2[:], in1=sin_o[:], op=OP.mult)
        v.tensor_scalar(out=sin_o[:], in0=sin_o[:], scalar1=1.0,
                        scalar2=-1.0 / 6.0, op0=OP.mult, op1=OP.add)
        v.tensor_tensor(out=sin_o[:], in0=t2[:], in1=sin_o[:], op=OP.mult)
        v.tensor_scalar(out=sin_o[:], in0=sin_o[:], scalar1=1.0,
                        scalar2=1.0, op0=OP.mult, op1=OP.add)
        v.tensor_tensor(out=sin_o[:], in0=ang_ap, in1=sin_o[:], op=OP.mult)
        v.tensor_scalar(out=cos_o[:], in0=t2[:], scalar1=1.0 / 40320.0,
                        scalar2=-1.0 / 720.0, op0=OP.mult, op1=OP.add)
        v.tensor_tensor(out=cos_o[:], in0=t2[:], in1=cos_o[:], op=OP.mult)
        v.tensor_scalar(out=cos_o[:], in0=cos_o[:], scalar1=1.0,
                        scalar2=1.0 / 24.0, op0=OP.mult, op1=OP.add)
        v.tensor_tensor(out=cos_o[:], in0=t2[:], in1=cos_o[:], op=OP.mult)
        v.tensor_scalar(out=cos_o[:], in0=cos_o[:], scalar1=1.0,
                        scalar2=-0.5, op0=OP.mult, op1=OP.add)
        v.tensor_tensor(out=cos_o[:], in0=t2[:], in1=cos_o[:], op=OP.mult)
        v.tensor_scalar(out=cos_o[:], in0=cos_o[:], scalar1=1.0,
                        scalar2=1.0, op0=OP.mult, op1=OP.add)

    with tile.TileContext(nc) as tc:
        with (
            tc.tile_pool(name="const", bufs=1) as cp,
            tc.tile_pool(name="sb", bufs=1) as sb,
            tc.tile_pool(name="ps", bufs=1, space="PSUM") as psp,
            tc.tile_pool(name="dram", bufs=1, space="DRAM") as dp,
        ):
            V = nc.vector

            # ------- constants / weights -------
            iota_i = cp.tile([P, P], i32, tag="iota_i")
            nc.gpsimd.iota(iota_i[:], pattern=[[1, P]], base=0, channel_multiplier=0)
            iota_f = cp.tile([P, P], f32, tag="iota_f")
            V.tensor_copy(iota_f[:], iota_i[:])
            ident = cp.tile([P, P], f32, tag="ident")
            make_identity(nc, ident[:])

            w1_t = cp.tile([17, 128], f16, tag="w1")
            nc.sync.dma_start(out=w1_t[:], in_=w1cat[:])
            w2bd_t = cp.tile([128, 72], f32, tag="w2bd")
            nc.sync.dma_start(out=w2bd_t[:], in_=w2bd[:])
            wnode_t = cp.tile([64, 65], f32, tag="wnode")
            nc.sync.dma_start(out=wnode_t[:], in_=wnode[:])
            wsc_t = cp.tile([16, 40], f32, tag="wsc")
            nc.sync.dma_start(out=wsc_t[:], in_=wsc[:])
            w2n_t = cp.tile([56, 64], f32, tag="w2n")
            nc.sync.dma_start(out=w2n_t[:], in_=w2n[:])
            w2b_t = cp.tile([40, 9], f32, tag="w2b")
            nc.sync.dma_start(out=w2b_t[:], in_=w2b[:])

            xn_h = sb.tile([16, P], f16, tag="xn_h")
            xn32 = sb.tile([16, P], f32, tag="xn32")
            an_t = cp.tile([P, NBLK], f32, tag="an")
            nc.sync.dma_start(out=an_t[:], in_=an[:])

            # ------- static SBUF workspaces -------
            es_u8 = sb.tile([16, 512], u8, tag="es_u8")
            es_s = cp.tile([17, 512], f16, tag="es_s")
            V.memset(es_s[:], 1.0)
            blob_h = sb.tile([P, 4, 5], f16, tag="blob_h")
            blob32 = sb.tile([P, 4, 5], f32, tag="blob32")
            xs_h = sb.tile([P, 4, 16], f16, tag="xs_h")
            xs32 = sb.tile([P, 4, 16], f32, tag="xs32")
            sg_s = sb.tile([P, 512], f32, tag="sg")
            h_s = sb.tile([P, 512], f32, tag="h")
            w_sb = sb.tile([P, 4, 72], f32, tag="w_sb")
            tmp16a = sb.tile([P, 4, 16], f32, tag="tmp16a")
            tmp16b = sb.tile([P, 4, 16], f32, tag="tmp16b")
            ef_s = sb.tile([P, 4, 64], f32, tag="ef")
            ind_s = sb.tile([P, P], f32, tag="ind")
            mid_sb = sb.tile([P, 64], f32, tag="mid_sb")
            tpc_s = sb.tile([P, P], f32, tag="tpc")      # copies of transposes
            y_a = sb.tile([P, 65], f32, tag="y_a")
            sc_a = sb.tile([P, 40], f32, tag="sc_a")
            trig = sb.tile([P, 4], f32, tag="trig")      # t2, sin, cos
            y40 = sb.tile([P, 40], f32, tag="y40")
            sgm = sb.tile([P, 32], f32, tag="sgm")
            h_blk = sb.tile([P, 56], f32, tag="h_blk")
            y01_sb = sb.tile([P, 64], f32, tag="y01_sb")
            # L2 workspaces
            w2_t2 = sb.tile([P, 4, 40], f32, tag="w2_t2")
            idx_t = sb.tile([P, 4], i32, tag="idx_t")
            g_s = sb.tile([P, 4, 64], f32, tag="g_s")
            ef2_s = sb.tile([P, 4, 40], f32, tag="ef2")
            tmp8a = sb.tile([P, 4, 8], f32, tag="tmp8a")
            tmp8b = sb.tile([P, 4, 8], f32, tag="tmp8b")
            mid2_sb = sb.tile([P, 40], f32, tag="mid2_sb")
            sc2_sb = sb.tile([P, 8], f32, tag="sc2_sb")
            y2a = sb.tile([P, 9], f32, tag="y2a")
            fo = sb.tile([P, 8], f32, tag="fo")

            # ------- PSUM tiles (8 banks) -------
            h_ps = psp.tile([P, 512], f32, tag="h_ps")
            w_ps = psp.tile([P, 72], f32, tag="w_ps")
            mid_ps = psp.tile([P, 64], f32, tag="mid_ps")
            tp_ps = psp.tile([P, P], f32, tag="tp_ps")
            nm_ps = psp.tile([P, 128], f32, tag="nm_ps")
            mid2_ps = psp.tile([P, 40], f32, tag="mid2_ps")

            # ------- DRAM scratch -------
            w2scr = dp.tile([NSUB * P, 40], f32, tag="w2scr")
            y01_loc = dp.tile([NODES_CORE, 64], f32, tag="y01_loc")

            # ================= L1 =================
            with tc.For_i(0, NBLK) as b:
                for u in range(SUPB):
                    nc.sync.dma_start(
                        out=es_u8[:], in_=es_t[:, ds(b * SLOTS_BLK + u * 512, 512)])
                    V.tensor_copy(es_s[0:16, :], es_u8[:])
                    nc.sync.dma_start(
                        out=blob_h[:],
                        in_=blob[:, ds((b * SUPB + u) * 4 * 5, 4 * 5)])
                    V.tensor_copy(blob32[:], blob_h[:])
                    nc.sync.dma_start(
                        out=idx_t[:], in_=srcidx[:, ds((b * SUPB + u) * 4, 4)])
                    for j in range(4):
                        nc.gpsimd.indirect_dma_start(
                            out=xs_h[:, j, :], out_offset=None,
                            in_=xfg[:],
                            in_offset=bass.IndirectOffsetOnAxis(
                                ap=idx_t[:, j:j + 1], axis=0))
                    V.tensor_copy(xs32[:], xs_h[:])
                    # MLP stage 1: h = silu(es^T @ w1cat)  -> [128 hf, 512 e]
                    nc.tensor.matmul(h_ps[:], lhsT=w1_t[:], rhs=es_s[:],
                                     start=True, stop=True)
                    nc.scalar.activation(sg_s[:], h_ps[:], AF.Sigmoid)
                    V.tensor_tensor(out=h_s[:], in0=h_ps[:], in1=sg_s[:], op=OP.mult)
                    # MLP stage 2 per subtile: w|w2 [128 e, 72]
                    for j in range(4):
                        nc.tensor.matmul(w_ps[:], lhsT=h_s[:, j * 128:(j + 1) * 128],
                                         rhs=w2bd_t[:], start=True, stop=True)
                        V.tensor_copy(w_sb[:, j, :], w_ps[:])
                    # stash w2 for L2
                    nc.sync.dma_start(
                        out=w2scr[ds((b * SUPB + u) * 4 * P, 4 * P), :],
                        in_=w_sb[:, :, 32:72])
                    # TP: ef [128 e, 64]
                    V.tensor_tensor(out=tmp16a[:], in0=w_sb[:, :, 0:16],
                                    in1=xs32[:], op=OP.mult)
                    V.tensor_tensor(out=ef_s[:, :, 0:16], in0=tmp16a[:],
                                    in1=blob32[:, :, 0:1].to_broadcast([P, 4, 16]),
                                    op=OP.mult)
                    V.tensor_tensor(out=tmp16b[:], in0=w_sb[:, :, 16:32],
                                    in1=xs32[:], op=OP.mult)
                    for c in range(3):
                        V.tensor_tensor(
                            out=ef_s[:, :, 16 + c:64:3], in0=tmp16b[:],
                            in1=blob32[:, :, 1 + c:2 + c].to_broadcast([P, 4, 16]),
                            op=OP.mult)
                    # scatter via indicator matmuls
                    for j in range(4):
                        V.tensor_tensor(
                            out=ind_s[:],
                            in0=blob32[:, j, 4:5].to_broadcast([P, P]),
                            in1=iota_f[:], op=OP.is_equal)
                        nc.tensor.matmul(mid_ps[:], lhsT=ind_s[:], rhs=ef_s[:, j, :],
                                         start=(u == 0 and j == 0),
                                         stop=(u == SUPB - 1 and j == 3))
                # ---- node math L1 ----
                nc.scalar.copy(mid_sb[:], mid_ps[:])
                if debug:
                    nc.sync.dma_start(out=mid_dbg[ds(b * P, P), :], in_=mid_sb[:])
                nc.tensor.transpose(out=tp_ps[0:64, :], in_=mid_sb[:], identity=ident[:])
                V.tensor_copy(tpc_s[0:64, :], tp_ps[0:64, :])
                nc.tensor.matmul(nm_ps[:, 0:65], lhsT=tpc_s[0:64, :], rhs=wnode_t[:],
                                 start=True, stop=True)
                V.tensor_tensor(out=y_a[:],
                                in0=nm_ps[:, 0:65],
                                in1=an_t[:, ds(b, 1)].to_broadcast([P, 65]),
                                op=OP.mult)
                nc.sync.dma_start(out=xn_h[:], in_=xnt[:, ds(b * P, P)])
                V.tensor_copy(xn32[:], xn_h[:])
                nc.tensor.matmul(nm_ps[:, 0:40], lhsT=xn32[:],
                                 rhs=wsc_t[:], start=True, stop=True)
                V.tensor_tensor(out=sc_a[:], in0=nm_ps[:, 0:40],
                                in1=an_t[:, ds(b, 1)].to_broadcast([P, 40]),
                                op=OP.mult)
                poly_sincos(V, trig[:, 0:1], trig[:, 1:2], trig[:, 2:3],
                            y_a[:, 64:65])
                # y40 = cos*sc_a + sin*conv_a
                V.tensor_tensor(out=y40[:], in0=sc_a[:],
                                in1=trig[:, 2:3].to_broadcast([P, 40]), op=OP.mult)
                V.tensor_tensor(out=sc_a[:], in0=y_a[:, 0:40],
                                in1=trig[:, 1:2].to_broadcast([P, 40]), op=OP.mult)
                V.tensor_tensor(out=y40[:], in0=y40[:], in1=sc_a[:], op=OP.add)
                # h block
                nc.scalar.activation(sgm[:], y40[:, 0:32], AF.Sigmoid)
                V.tensor_tensor(out=h_blk[:, 0:32], in0=y40[:, 0:32], in1=sgm[:],
                                op=OP.mult)
                nc.scalar.activation(sgm[:, 0:8], y40[:, 32:40], AF.Sigmoid)
                for c in range(3):
                    V.tensor_tensor(out=h_blk[:, 32 + c:56:3],
                                    in0=y_a[:, 40 + c:64:3], in1=sgm[:, 0:8],
                                    op=OP.mult)
                # y0|y1|sc2
                nc.tensor.transpose(out=tp_ps[0:56, :], in_=h_blk[:], identity=ident[:])
                V.tensor_copy(tpc_s[0:56, :], tp_ps[0:56, :])
                nc.tensor.matmul(nm_ps[:, 0:64], lhsT=tpc_s[0:56, :], rhs=w2n_t[:],
                                 start=True, stop=True)
                V.tensor_tensor(out=y01_sb[:], in0=nm_ps[:, 0:64],
                                in1=an_t[:, ds(b, 1)].to_broadcast([P, 64]),
                                op=OP.mult)
                nc.sync.dma_start(out=y01_loc[ds(b * P, P), :], in_=y01_sb[:])

            # ================= AllGather =================
            nc.gpsimd.collective_compute(
                "AllGather",
                OP.bypass,
                replica_groups=[list(range(NC))],
                ins=[y01_loc[:]],
                outs=[y01_all[:]],
            )

            # ================= L2 =================
            with tc.For_i(0, NBLK) as b:
                nc.sync.dma_start(out=sc2_sb[:],
                                  in_=y01_loc[ds(b * P, P), 56:64])
                if debug:
                    nc.sync.dma_start(out=y01_dbg[ds(b * P, P), :],
                                      in_=y01_loc[ds(b * P, P), :])
                for u in range(SUPB):
                    nc.sync.dma_start(
                        out=blob_h[:],
                        in_=blob[:, ds((b * SUPB + u) * 4 * 5, 4 * 5)])
                    V.tensor_copy(blob32[:], blob_h[:])
                    nc.sync.dma_start(
                        out=w2_t2[:],
                        in_=w2scr[ds((b * SUPB + u) * 4 * P, 4 * P), :])
                    nc.sync.dma_start(
                        out=idx_t[:], in_=srcidx[:, ds((b * SUPB + u) * 4, 4)])
                    for j in range(4):
                        nc.gpsimd.indirect_dma_start(
                            out=g_s[:, j, :], out_offset=None,
                            in_=y01_all[:],
                            in_offset=bass.IndirectOffsetOnAxis(
                                ap=idx_t[:, j:j + 1], axis=0))
                    # ef2
                    V.tensor_tensor(out=ef2_s[:, :, 0:32], in0=w2_t2[:, :, 0:32],
                                    in1=g_s[:, :, 0:32], op=OP.mult)
                    V.tensor_tensor(out=ef2_s[:, :, 0:32], in0=ef2_s[:, :, 0:32],
                                    in1=blob32[:, :, 0:1].to_broadcast([P, 4, 32]),
                                    op=OP.mult)
                    V.tensor_tensor(out=tmp8a[:], in0=g_s[:, :, 32:56:3],
                                    in1=blob32[:, :, 1:2].to_broadcast([P, 4, 8]),
                                    op=OP.mult)
                    V.tensor_tensor(out=tmp8b[:], in0=g_s[:, :, 33:56:3],
                                    in1=blob32[:, :, 2:3].to_broadcast([P, 4, 8]),
                                    op=OP.mult)
                    V.tensor_tensor(out=tmp8a[:], in0=tmp8a[:], in1=tmp8b[:],
                                    op=OP.add)
                    V.tensor_tensor(out=tmp8b[:], in0=g_s[:, :, 34:56:3],
                                    in1=blob32[:, :, 3:4].to_broadcast([P, 4, 8]),
                                    op=OP.mult)
                    V.tensor_tensor(out=tmp8a[:], in0=tmp8a[:], in1=tmp8b[:],
                                    op=OP.add)
                    V.tensor_tensor(out=ef2_s[:, :, 32:40], in0=w2_t2[:, :, 32:40],
                                    in1=tmp8a[:], op=OP.mult)
                    for j in range(4):
                        V.tensor_tensor(
                            out=ind_s[:],
                            in0=blob32[:, j, 4:5].to_broadcast([P, P]),
                            in1=iota_f[:], op=OP.is_equal)
                        nc.tensor.matmul(mid2_ps[:], lhsT=ind_s[:],
                                         rhs=ef2_s[:, j, :],
                                         start=(u == 0 and j == 0),
                                         stop=(u == SUPB - 1 and j == 3))
                # ---- node math L2 ----
                nc.scalar.copy(mid2_sb[:], mid2_ps[:])
                if debug:
                    nc.sync.dma_start(out=mid2_dbg[ds(b * P, P), :], in_=mid2_sb[:])
                nc.tensor.transpose(out=tp_ps[0:40, :], in_=mid2_sb[:],
                                    identity=ident[:])
                V.tensor_copy(tpc_s[0:40, :], tp_ps[0:40, :])
                nc.tensor.matmul(nm_ps[:, 0:9], lhsT=tpc_s[0:40, :], rhs=w2b_t[:],
                                 start=True, stop=True)
                V.tensor_tensor(out=y2a[:], in0=nm_ps[:, 0:9],
                                in1=an_t[:, ds(b, 1)].to_broadcast([P, 9]),
                                op=OP.mult)
                poly_sincos(V, trig[:, 0:1], trig[:, 1:2], trig[:, 2:3],
                            y2a[:, 8:9])
                V.tensor_tensor(out=fo[:], in0=sc2_sb[:],
                                in1=trig[:, 2:3].to_broadcast([P, 8]), op=OP.mult)
                V.tensor_tensor(out=y2a[:, 0:8], in0=y2a[:, 0:8],
                                in1=trig[:, 1:2].to_broadcast([P, 8]), op=OP.mult)
                V.tensor_tensor(out=fo[:], in0=fo[:], in1=y2a[:, 0:8], op=OP.add)
                nc.sync.dma_start(out=out_d[ds(b * P, P), :], in_=fo[:])

    nc.compile()
    return nc


def _prep_inputs(node_features, node_attr, edge_attr, edge_scalars,
                 sc1_w, lin1_w, fc1_w1, fc1_w2, lin2_w0, lin2_w1, lin3_w,
                 sc2_w, lin1b_w0, lin1b_w1, fc2_w1, fc2_w2, lin2b_w, lin3b_w,
                 edge_src, edge_dst):
    f = np.float32
    x = np.asarray(node_features, f)
    a = np.asarray(node_attr, f)
    ea = np.asarray(edge_attr, f)
    es = np.asarray(edge_scalars, f)
    src = np.asarray(edge_src).astype(np.int64)
    dst = np.asarray(edge_dst).astype(np.int64)

    inv_nn = 0.25
    S3 = np.sqrt(3.0)
    inv32, inv8, inv40 = 1 / np.sqrt(32.0), 1 / np.sqrt(8.0), 1 / np.sqrt(40.0)

    # ---- weights ----
    w1raw = np.concatenate([np.asarray(fc1_w1, f) / 4.0,
                            np.asarray(fc2_w1, f) / 4.0], axis=1)
    w1cat = np.zeros((17, 128), f)
    w1cat[0:16] = w1raw / 256.0
    w1cat[16] = (0.5 / 256.0) * w1raw.sum(axis=0)
    w1cat = w1cat.astype(np.float16)
    w2bd = np.zeros((128, 72), f)
    w2bd[0:64, 0:32] = np.asarray(fc1_w2, f) * (inv_nn / 8.0)
    w2bd[64:128, 32:72] = np.asarray(fc2_w2, f) * (inv_nn / 8.0)
    w2bd[64:128, 64:72] /= S3
    wnode = np.zeros((64, 65), f)
    wnode[0:16, 0:40] = np.asarray(lin2_w0, f) / 4.0
    wnode[0:16, 64] = 0.1 / 4.0 * np.asarray(lin3_w, f)[:, 0]
    l2w1 = np.asarray(lin2_w1, f) / 4.0
    for u in range(16):
        for w in range(8):
            for c in range(3):
                wnode[16 + 3 * u + c, 40 + 3 * w + c] = l2w1[u, w]
    wsc = (np.asarray(sc1_w, f) / 4.0).astype(f)
    w2n = np.zeros((56, 64), f)
    w2n[0:32, 0:32] = np.asarray(lin1b_w0, f) * inv32
    w2n[0:32, 56:64] = np.asarray(sc2_w, f) * inv32
    l1bw1 = np.asarray(lin1b_w1, f) * inv8
    for u in range(8):
        for w in range(8):
            for c in range(3):
                w2n[32 + 3 * u + c, 32 + 3 * w + c] = l1bw1[u, w]
    w2b = np.zeros((40, 9), f)
    w2b[:, 0:8] = np.asarray(lin2b_w, f) * inv40
    w2b[:, 8] = 0.1 * inv40 * np.asarray(lin3b_w, f)[:, 0]

    # ---- per-node ----
    xf_a = (x @ np.asarray(lin1_w, f)) / 4.0 * a     # [N,16]
    x_pad = np.zeros((N_PAD, 16), f)
    x_pad[:N] = x
    a_pad = np.zeros((N_PAD, 1), f)
    a_pad[:N] = a

    # ---- node rebalancing: deal nodes into degree-balanced 128-node blocks ----
    deg = np.bincount(dst, minlength=N_PAD)
    order = np.argsort(-deg, kind="stable")
    nbins = NC * NBLK
    loads = np.zeros(nbins, np.int64)
    new_of_old = np.empty(N_PAD, np.int64)
    for r in range(P):
        chunk = order[r * nbins:(r + 1) * nbins]
        bin_order = np.argsort(loads, kind="stable")
        new_of_old[chunk] = bin_order * P + r
        loads[bin_order] += deg[chunk]
    assert loads.max() <= SLOTS_BLK, f"block overflow after dealing: {loads.max()}"
    old_of_new = np.argsort(new_of_old, kind="stable")
    dst = new_of_old[dst]
    src = new_of_old[src]
    x_pad = x_pad[old_of_new]
    a_pad = a_pad[old_of_new]

    # ---- edge sort & slotting ----
    perm = np.argsort(dst, kind="stable")
    dst_s = dst[perm]
    src_s = src[perm]
    gb = (dst_s // P).astype(np.int64)              # global block 0..391
    blk_counts = np.bincount(gb, minlength=NC * NBLK)
    assert blk_counts.max() <= SLOTS_BLK, f"block overflow: {blk_counts.max()}"
    starts = np.zeros(NC * NBLK + 1, np.int64)
    np.cumsum(blk_counts, out=starts[1:])
    rank = np.arange(E, dtype=np.int64) - starts[gb]
    slot = gb * SLOTS_BLK + rank                     # global slot id

    S_ALL = NC * SLOTS_CORE
    es_pack = np.zeros((S_ALL, 16), np.uint8)
    es_pack[slot] = np.minimum(np.floor(es[perm] * 256.0), 255.0).astype(np.uint8)
    blob = np.zeros((S_ALL, 5), np.float16)
    blob[:, 4] = -1.0
    blob[slot, 0] = ea[perm, 0].astype(np.float16)
    blob[slot, 1:4] = ea[perm, 1:4].astype(np.float16)
    blob[slot, 4] = (dst_s - gb * P).astype(np.float16)
    srcidx = np.zeros((S_ALL,), np.int32)
    srcidx[slot] = src_s.astype(np.int32)

    xf_pad = np.zeros((N_PAD, 16), np.float16)
    xf_pad[:N] = xf_a.astype(np.float16)
    xf_pad = xf_pad[old_of_new]

    # global [NC*dim0, ...] arrays, matching shard_map's concat layout
    cat = {
        "es_t": np.ascontiguousarray(
            es_pack.reshape(NC, SLOTS_CORE, 16).transpose(0, 2, 1)
        ).reshape(NC * 16, SLOTS_CORE),
        "blob": np.ascontiguousarray(
            blob.reshape(NC, NSUB, P, 5).transpose(0, 2, 1, 3)
        ).reshape(NC * P, NSUB * 5),
        "srcidx": np.ascontiguousarray(
            srcidx.reshape(NC, NSUB, P).transpose(0, 2, 1)
        ).reshape(NC * P, NSUB),
        "xfg": np.tile(xf_pad, (NC, 1)),
        "xnt": np.ascontiguousarray(
            x_pad.reshape(NC, NODES_CORE, 16).transpose(0, 2, 1)
        ).astype(np.float16).reshape(NC * 16, NODES_CORE),
        "an": np.ascontiguousarray(
            a_pad[:, 0].reshape(NC, NBLK, P).transpose(0, 2, 1)
        ).reshape(NC * P, NBLK),
        "w1cat": np.tile(w1cat, (NC, 1)),
        "w2bd": np.tile(w2bd, (NC, 1)),
        "wnode": np.tile(wnode, (NC, 1)),
        "wsc": np.tile(wsc, (NC, 1)),
        "w2n": np.tile(w2n, (NC, 1)),
        "w2b": np.tile(w2b, (NC, 1)),
    }
    return cat, new_of_old


def _get_runner():
    """Build (once) and cache a jitted SPMD callable for the compiled Bass
    module, mirroring bass2jax.run_bass_via_pjrt. Re-using it across calls
    skips the per-call jax retrace + XLA/walrus compile."""
    if "runner" in _CACHED:
        return _CACHED["runner"]
    import jax
    from jax.experimental.shard_map import shard_map
    from jax.sharding import Mesh, PartitionSpec
    from concourse import bass2jax, mybir
    import concourse.bass  # noqa: F401

    bass2jax.install_neuronx_cc_hook()
    nc = _CACHED["nc"]
    assert nc.dbg_addr is None
    partition_name = nc.partition_id_tensor.name if nc.partition_id_tensor else None

    in_names, out_names, out_avals, zero_shapes = [], [], [], []
    for alloc in nc.m.functions[0].allocations:
        if not isinstance(alloc, mybir.MemoryLocationSet):
            continue
        name = alloc.memorylocations[0].name
        if alloc.kind == "ExternalInput":
            if name != partition_name:
                in_names.append(name)
        elif alloc.kind == "ExternalOutput":
            shape = tuple(alloc.tensor_shape)
            dtype = mybir.dt.np(alloc.dtype)
            out_names.append(name)
            out_avals.append(jax.core.ShapedArray(shape, dtype))
            zero_shapes.append((shape, dtype))
    n_params = len(in_names)
    all_names = list(in_names) + list(out_names)
    if partition_name is not None:
        all_names.append(partition_name)
    donate = tuple(range(n_params, n_params + len(out_names)))

    def _body(*args):
        operands = list(args)
        if partition_name is not None:
            operands.append(bass2jax.partition_id_tensor())
        outs = bass2jax._bass_exec_p.bind(
            *operands,
            out_avals=tuple(out_avals),
            in_names=tuple(all_names),
            out_names=tuple(out_names),
            lowering_input_output_aliases=(),
            sim_require_finite=True,
            sim_require_nnan=True,
            nc=nc,
        )
        return tuple(outs)

    devices = jax.devices()[:NC]
    mesh = Mesh(np.asarray(devices), ("core",))
    in_specs = (PartitionSpec("core"),) * (n_params + len(out_names))
    out_specs = (PartitionSpec("core"),) * len(out_names)
    sharded = jax.jit(
        shard_map(_body, mesh=mesh, in_specs=in_specs, out_specs=out_specs,
                  check_rep=False),
        donate_argnums=donate, keep_unused=True)
    _CACHED["runner"] = (sharded, in_names, out_names, zero_shapes)
    return _CACHED["runner"]


def _run(cat):
    sharded, in_names, out_names, zero_shapes = _get_runner()
    concat_in = [cat[nm] for nm in in_names]
    concat_zeros = [
        np.zeros((NC * s[0], *s[1:]), dt) for s, dt in zero_shapes]
    out_arrs = sharded(*concat_in, *concat_zeros)
    return [
        {nm: np.asarray(out_arrs[i]).reshape(NC, *zero_shapes[i][0])[c]
         for i, nm in enumerate(out_names)}
        for c in range(NC)]


def _split_maps(cat, nc):
    """Per-core in_maps view of the global arrays (debug path)."""
    import concourse.mybir as mybir
    dims = {}
    for alloc in nc.m.functions[0].allocations:
        if isinstance(alloc, mybir.MemoryLocationSet) and alloc.kind == "ExternalInput":
            dims[alloc.memorylocations[0].name] = tuple(alloc.tensor_shape)
    return [
        {nm: cat[nm].reshape(NC, *dims[nm])[k] for nm in cat}
        for k in range(NC)]


def _warmup():
    """Pay bass-build + walrus compile + NEFF load + device/comm init once at
    import time with synthetic (structurally valid) inputs, so the real call
    runs the warm path."""
    _CACHED["nc"] = _build_bass(debug=False)
    f = np.float32
    dummy = {
        "node_features": np.zeros((N, 16), f),
        "node_attr": np.ones((N, 1), f),
        "edge_attr": np.zeros((E, 4), f),
        "edge_scalars": np.zeros((E, 16), f),
        "sc1_w": np.zeros((16, 40), f), "lin1_w": np.zeros((16, 16), f),
        "fc1_w1": np.zeros((16, 64), f), "fc1_w2": np.zeros((64, 32), f),
        "lin2_w0": np.zeros((16, 40), f), "lin2_w1": np.zeros((16, 8), f),
        "lin3_w": np.zeros((16, 1), f),
        "sc2_w": np.zeros((32, 8), f), "lin1b_w0": np.zeros((32, 32), f),
        "lin1b_w1": np.zeros((8, 8), f), "fc2_w1": np.zeros((16, 64), f),
        "fc2_w2": np.zeros((64, 40), f), "lin2b_w": np.zeros((40, 8), f),
        "lin3b_w": np.zeros((40, 1), f),
        "edge_src": (np.arange(E, dtype=np.int64) * 7919) % N,
        "edge_dst": np.arange(E, dtype=np.int64) % N,
    }
    cat, _ = _prep_inputs(**dummy)
    _run(cat)


try:
    _warmup()
except Exception:
    _CACHED.pop("runner", None)


def kernel(node_features, node_attr, edge_attr, edge_scalars,
           sc1_w, lin1_w, fc1_w1, fc1_w2, lin2_w0, lin2_w1, lin3_w,
           sc2_w, lin1b_w0, lin1b_w1, fc2_w1, fc2_w2, lin2b_w, lin3b_w,
           edge_src, edge_dst, _debug=False):
    global LAST_EXEC_NS
    from concourse.bass_utils import run_bass_kernel_spmd

    key = "nc_dbg" if _debug else "nc"
    if key not in _CACHED:
        _CACHED[key] = _build_bass(debug=_debug)
    nc = _CACHED[key]

    cat, new_of_old = _prep_inputs(
        node_features, node_attr, edge_attr, edge_scalars,
        sc1_w, lin1_w, fc1_w1, fc1_w2, lin2_w0, lin2_w1, lin3_w,
        sc2_w, lin1b_w0, lin1b_w1, fc2_w1, fc2_w2, lin2b_w, lin3b_w,
        edge_src, edge_dst)
    _CACHED["new_of_old"] = new_of_old

    t0 = time.perf_counter()
    if _debug:
        results = run_bass_kernel_spmd(nc, _split_maps(cat, nc),
                                       list(range(NC))).results
    else:
        try:
            results = _run(cat)
        except Exception:
            results = run_bass_kernel_spmd(nc, _split_maps(cat, nc),
                                           list(range(NC))).results
    t1 = time.perf_counter()
    LAST_EXEC_NS = int((t1 - t0) * 1e9)

    out = np.empty((N_PAD, 8), np.float32)
    for k in range(NC):
        out[k * NODES_CORE:(k + 1) * NODES_CORE] = np.asarray(results[k]["out_d"])
    if _debug:
        dbg = {nm: np.concatenate(
            [np.asarray(results[k][nm]) for k in range(NC)], axis=0)[new_of_old]
            for nm in ("mid_dbg", "y01_dbg", "mid2_dbg")}
        return out[new_of_old[:N]], dbg
    return out[new_of_old[:N]]


# revision 12
# speedup vs baseline: 1.0502x; 1.0502x over previous
"""Trainium2 kernel for nn_MessagePassing_22497038696556 (gnn_message_passing).

Single-NEFF, fully-on-device design (edge-parallel over 8 cores, dst-sharded):
  - Nodes are dealt degree-balanced into 128-node blocks (49 blocks/core,
    50176 total incl. pad; every block carries 2038-2044 edges), so each
    block fits exactly 4 "supertiles" of 512 edge slots (2048 slots, <1%%
    pad). Edges are sorted by the permuted dst; pad slots have dst_rel = -1.
    The host un-permutes the final output rows.
  - Per core, one Bass kernel does EVERYTHING:
      L1: gather xf[src] via indirect DMA from a replicated fp16 table ->
          edge MLP (fp16 matmuls) -> per-edge TP -> segment-sum into the
          node dim via iota/is_equal indicator matmuls accumulating in PSUM
          -> layer-1 node math (transpose + stacked-weight matmuls, Taylor
          sin/cos) -> y0|y1|sc2 table [6272, 64] per core.
      One AllGather (NeuronLink) -> y01 table for all 50176 nodes.
      L2: per-edge gather of y01[src] via indirect DMA -> TP2 -> indicator
          matmul segment-sum -> layer-2 node math -> out [6272, 8].
  - Host only sorts/packs inputs and concatenates the 8 outputs. Edge
    scalars travel as uint8 fixed-point (exact-integer fp16 matmul with a
    ones-row bias fold); other per-edge data as fp16.
  - Import-time warmup with synthetic inputs pays bass-build + walrus
    compile + NEFF load + comm init once; the jitted SPMD callable is
    cached, so kernel() itself is transfers (~39MB in / 1.6MB out through
    the PJRT path) + ~0.1s device exec.
"""

import time
import numpy as np

N = 50000
E = 800000
NC = 8
P = 128
NBLK = 49                      # node blocks per core
NODES_CORE = NBLK * P          # 6272
N_PAD = NC * NODES_CORE        # 50176
SUPB = 4                       # supertiles per block (nodes are re-dealt into degree-balanced blocks)
SLOTS_BLK = SUPB * 512         # 2048
TOTSUP = NBLK * SUPB           # 196 supertiles per core
SLOTS_CORE = NBLK * SLOTS_BLK  # 100352
NSUB = TOTSUP * 4              # 784 subtiles per core

LAST_EXEC_NS = None
_CACHED = {}


def _build_bass(debug=False):
    import concourse.bass as bass
    import concourse.mybir as mybir
    import concourse.tile as tile
    from concourse import bacc
    from concourse.bass import ds
    from concourse.masks import make_identity

    f32 = mybir.dt.float32
    f16 = mybir.dt.float16
    i32 = mybir.dt.int32
    u8 = mybir.dt.uint8
    AF = mybir.ActivationFunctionType
    OP = mybir.AluOpType

    nc = bacc.Bacc(None, target_bir_lowering=False, num_devices=NC)

    # ---- inputs ----
    es_t = nc.dram_tensor("es_t", [16, SLOTS_CORE], u8, kind="ExternalInput")
    blob = nc.dram_tensor("blob", [P, NSUB * 5], f16, kind="ExternalInput")
    xfg = nc.dram_tensor("xfg", [N_PAD, 16], f16, kind="ExternalInput")
    srcidx = nc.dram_tensor("srcidx", [P, NSUB], i32, kind="ExternalInput")
    xnt = nc.dram_tensor("xnt", [16, NODES_CORE], f16, kind="ExternalInput")
    an = nc.dram_tensor("an", [P, NBLK], f32, kind="ExternalInput")
    w1cat = nc.dram_tensor("w1cat", [17, 128], f16, kind="ExternalInput")
    w2bd = nc.dram_tensor("w2bd", [128, 72], f32, kind="ExternalInput")
    wnode = nc.dram_tensor("wnode", [64, 65], f32, kind="ExternalInput")
    wsc = nc.dram_tensor("wsc", [16, 40], f32, kind="ExternalInput")
    w2n = nc.dram_tensor("w2n", [56, 64], f32, kind="ExternalInput")
    w2b = nc.dram_tensor("w2b", [40, 9], f32, kind="ExternalInput")
    # ---- outputs ----
    out_d = nc.dram_tensor("out_d", [NODES_CORE, 8], f32, kind="ExternalOutput")
    if debug:
        mid_dbg = nc.dram_tensor("mid_dbg", [NODES_CORE, 64], f32, kind="ExternalOutput")
        y01_dbg = nc.dram_tensor("y01_dbg", [NODES_CORE, 64], f32, kind="ExternalOutput")
        mid2_dbg = nc.dram_tensor("mid2_dbg", [NODES_CORE, 40], f32, kind="ExternalOutput")

    y01_all = nc.dram_tensor("y01_all", [N_PAD, 64], f32, kind="Internal",
                             addr_space="Shared")

    def poly_sincos(v, t2, sin_o, cos_o, ang_ap):
        # t2 = ang^2; sin = ang*(1 + t2*(-1/6 + t2*(1/120 - t2/5040)))
        # cos = 1 + t2*(-1/2 + t2*(1/24 + t2*(-1/720 + t2/40320)))
        v.tensor_tensor(out=t2[:], in0=ang_ap, in1=ang_ap, op=OP.mult)
        v.tensor_scalar(out=sin_o[:], in0=t2[:], scalar1=-1.0 / 5040.0,
                        scalar2=1.0 / 120.0, op0=OP.mult, op1=OP.add)
        v.tensor_tensor(out=sin_o[:], in0=t2[:], in1=sin_o[:], op=OP.mult)
        v.tensor_scalar(out=sin_o[:], in0=sin_o[:], scalar1=1.0,
                        scalar2=-1.0 / 6.0, op0=OP.mult, op1=OP.add)
        v.tensor_tensor(out=sin_o[:], in0=t2[:], in1=sin_o[:], op=OP.mult)
        v.tensor_scalar(out=sin_o[:], in0=sin_o[:], scalar1=1.0,
                        scalar2=1.0, op0=OP.mult, op1=OP.add)
        v.tensor_tensor(out=sin_o[:], in0=ang_ap, in1=sin_o[:], op=OP.mult)
        v.tensor_scalar(out=cos_o[:], in0=t2[:], scalar1=1.0 / 40320.0,
                        scalar2=-1.0 / 720.0, op0=OP.mult, op1=OP.add)
        v.tensor_tensor(out=cos_o[:], in0=t2[:], in1=cos_o[:], op=OP.mult)
        v.tensor_scalar(out=cos_o[:], in0=cos_o[:], scalar1=1.0,
                        scalar2=1.0 / 24.0, op0=OP.mult, op1=OP.add)
        v.tensor_tensor(out=cos_o[:], in0=t2[:], in1=cos_o[:], op=OP.mult)
        v.tensor_scalar(out=cos_o[:], in0=cos_o[:], scalar1=1.0,
                        scalar2=-0.5, op0=OP.mult, op1=OP.add)
        v.tensor_tensor(out=cos_o[:], in0=t2[:], in1=cos_o[:], op=OP.mult)
        v.tensor_scalar(out=cos_o[:], in0=cos_o[:], scalar1=1.0,
                        scalar2=1.0, op0=OP.mult, op1=OP.add)

    with tile.TileContext(nc) as tc:
        with (
            tc.tile_pool(name="const", bufs=1) as cp,
            tc.tile_pool(name="sb", bufs=1) as sb,
            tc.tile_pool(name="ps", bufs=1, space="PSUM") as psp,
            tc.tile_pool(name="dram", bufs=1, space="DRAM") as dp,
        ):
            V = nc.vector

            # ------- constants / weights -------
            iota_i = cp.tile([P, P], i32, tag="iota_i")
            nc.gpsimd.iota(iota_i[:], pattern=[[1, P]], base=0, channel_multiplier=0)
            iota_f = cp.tile([P, P], f32, tag="iota_f")
            V.tensor_copy(iota_f[:], iota_i[:])
            ident = cp.tile([P, P], f32, tag="ident")
            make_identity(nc, ident[:])

            w1_t = cp.tile([17, 128], f16, tag="w1")
            nc.sync.dma_start(out=w1_t[:], in_=w1cat[:])
            w2bd_t = cp.tile([128, 72], f32, tag="w2bd")
            nc.sync.dma_start(out=w2bd_t[:], in_=w2bd[:])
            wnode_t = cp.tile([64, 65], f32, tag="wnode")
            nc.sync.dma_start(out=wnode_t[:], in_=wnode[:])
            wsc_t = cp.tile([16, 40], f32, tag="wsc")
            nc.sync.dma_start(out=wsc_t[:], in_=wsc[:])
            w2n_t = cp.tile([56, 64], f32, tag="w2n")
            nc.sync.dma_start(out=w2n_t[:], in_=w2n[:])
            w2b_t = cp.tile([40, 9], f32, tag="w2b")
            nc.sync.dma_start(out=w2b_t[:], in_=w2b[:])

            xn_h = sb.tile([16, P], f16, tag="xn_h")
            xn32 = sb.tile([16, P], f32, tag="xn32")
            an_t = cp.tile([P, NBLK], f32, tag="an")
            nc.sync.dma_start(out=an_t[:], in_=an[:])

            # ------- static SBUF workspaces -------
            es_u8 = sb.tile([16, 512], u8, tag="es_u8")
            es_s = cp.tile([17, 512], f16, tag="es_s")
            V.memset(es_s[:], 1.0)
            blob_h = sb.tile([P, 4, 5], f16, tag="blob_h")
            blob32 = sb.tile([P, 4, 5], f32, tag="blob32")
            xs_h = sb.tile([P, 4, 16], f16, tag="xs_h")
            xs32 = sb.tile([P, 4, 16], f32, tag="xs32")
            sg_s = sb.tile([P, 512], f32, tag="sg")
            h_s = sb.tile([P, 512], f32, tag="h")
            w_sb = sb.tile([P, 4, 72], f32, tag="w_sb")
            tmp16a = sb.tile([P, 4, 16], f32, tag="tmp16a")
            tmp16b = sb.tile([P, 4, 16], f32, tag="tmp16b")
            ef_s = sb.tile([P, 4, 64], f32, tag="ef")
            ind_s = sb.tile([P, P], f32, tag="ind")
            mid_sb = sb.tile([P, 64], f32, tag="mid_sb")
            tpc_s = sb.tile([P, P], f32, tag="tpc")      # copies of transposes
            y_a = sb.tile([P, 65], f32, tag="y_a")
            sc_a = sb.tile([P, 40], f32, tag="sc_a")
            trig = sb.tile([P, 4], f32, tag="trig")      # t2, sin, cos
            y40 = sb.tile([P, 40], f32, tag="y40")
            sgm = sb.tile([P, 32], f32, tag="sgm")
            h_blk = sb.tile([P, 56], f32, tag="h_blk")
            y01_sb = sb.tile([P, 64], f32, tag="y01_sb")
            # L2 workspaces
            w2_t2 = sb.tile([P, 4, 40], f32, tag="w2_t2")
            idx_t = sb.tile([P, 4], i32, tag="idx_t")
            g_s = sb.tile([P, 4, 64], f32, tag="g_s")
            ef2_s = sb.tile([P, 4, 40], f32, tag="ef2")
            tmp8a = sb.tile([P, 4, 8], f32, tag="tmp8a")
            tmp8b = sb.tile([P, 4, 8], f32, tag="tmp8b")
            mid2_sb = sb.tile([P, 40], f32, tag="mid2_sb")
            sc2_sb = sb.tile([P, 8], f32, tag="sc2_sb")
            y2a = sb.tile([P, 9], f32, tag="y2a")
            fo = sb.tile([P, 8], f32, tag="fo")

            # ------- PSUM tiles (8 banks) -------
            h_ps = psp.tile([P, 512], f32, tag="h_ps")
            w_ps = psp.tile([P, 72], f32, tag="w_ps")
            mid_ps = psp.tile([P, 64], f32, tag="mid_ps")
            tp_ps = psp.tile([P, P], f32, tag="tp_ps")
            nm_ps = psp.tile([P, 128], f32, tag="nm_ps")
            mid2_ps = psp.tile([P, 40], f32, tag="mid2_ps")

            # ------- DRAM scratch -------
            w2scr = dp.tile([NSUB * P, 40], f32, tag="w2scr")
            y01_loc = dp.tile([NODES_CORE, 64], f32, tag="y01_loc")

            # ================= L1 =================
            with tc.For_i(0, NBLK) as b:
                for u in range(SUPB):
                    nc.sync.dma_start(
                        out=es_u8[:], in_=es_t[:, ds(b * SLOTS_BLK + u * 512, 512)])
                    V.tensor_copy(es_s[0:16, :], es_u8[:])
                    nc.sync.dma_start(
                        out=blob_h[:],
                        in_=blob[:, ds((b * SUPB + u) * 4 * 5, 4 * 5)])
                    V.tensor_copy(blob32[:], blob_h[:])
                    nc.sync.dma_start(
                        out=idx_t[:], in_=srcidx[:, ds((b * SUPB + u) * 4, 4)])
                    for j in range(4):
                        nc.gpsimd.indirect_dma_start(
                            out=xs_h[:, j, :], out_offset=None,
                            in_=xfg[:],
                            in_offset=bass.IndirectOffsetOnAxis(
                                ap=idx_t[:, j:j + 1], axis=0))
                    V.tensor_copy(xs32[:], xs_h[:])
                    # MLP stage 1: h = silu(es^T @ w1cat)  -> [128 hf, 512 e]
                    nc.tensor.matmul(h_ps[:], lhsT=w1_t[:], rhs=es_s[:],
                                     start=True, stop=True)
                    nc.scalar.activation(sg_s[:], h_ps[:], AF.Sigmoid)
                    V.tensor_tensor(out=h_s[:], in0=h_ps[:], in1=sg_s[:], op=OP.mult)
                    # MLP stage 2 per subtile: w|w2 [128 e, 72]
                    for j in range(4):
                        nc.tensor.matmul(w_ps[:], lhsT=h_s[:, j * 128:(j + 1) * 128],
                                         rhs=w2bd_t[:], start=True, stop=True)
                        V.tensor_copy(w_sb[:, j, :], w_ps[:])
                    # stash w2 for L2
                    nc.sync.dma_start(
                        out=w2scr[ds((b * SUPB + u) * 4 * P, 4 * P), :],
                        in_=w_sb[:, :, 32:72])
                    # TP: ef [128 e, 64]
                    V.tensor_tensor(out=tmp16a[:], in0=w_sb[:, :, 0:16],
                                    in1=xs32[:], op=OP.mult)
                    V.tensor_tensor(out=ef_s[:, :, 0:16], in0=tmp16a[:],
                                    in1=blob32[:, :, 0:1].to_broadcast([P, 4, 16]),
                                    op=OP.mult)
                    V.tensor_tensor(out=tmp16b[:], in0=w_sb[:, :, 16:32],
                                    in1=xs32[:], op=OP.mult)
                    for c in range(3):
                        V.tensor_tensor(
                            out=ef_s[:, :, 16 + c:64:3], in0=tmp16b[:],
                            in1=blob32[:, :, 1 + c:2 + c].to_broadcast([P, 4, 16]),
                            op=OP.mult)
                    # scatter via indicator matmuls
                    for j in range(4):
                        V.tensor_tensor(
                            out=ind_s[:],
                            in0=blob32[:, j, 4:5].to_broadcast([P, P]),
                            in1=iota_f[:], op=OP.is_equal)
                        nc.tensor.matmul(mid_ps[:], lhsT=ind_s[:], rhs=ef_s[:, j, :],
                                         start=(u == 0 and j == 0),
                                         stop=(u == SUPB - 1 and j == 3))
                # ---- node math L1 ----
                nc.scalar.copy(mid_sb[:], mid_ps[:])
                if debug:
                    nc.sync.dma_start(out=mid_dbg[ds(b * P, P), :], in_=mid_sb[:])
                nc.tensor.transpose(out=tp_ps[0:64, :], in_=mid_sb[:], identity=ident[:])
                V.tensor_copy(tpc_s[0:64, :], tp_ps[0:64, :])
                nc.tensor.matmul(nm_ps[:, 0:65], lhsT=tpc_s[0:64, :], rhs=wnode_t[:],
                                 start=True, stop=True)
                V.tensor_tensor(out=y_a[:],
                                in0=nm_ps[:, 0:65],
                                in1=an_t[:, ds(b, 1)].to_broadcast([P, 65]),
                                op=OP.mult)
                nc.sync.dma_start(out=xn_h[:], in_=xnt[:, ds(b * P, P)])
                V.tensor_copy(xn32[:], xn_h[:])
                nc.tensor.matmul(nm_ps[:, 0:40], lhsT=xn32[:],
                                 rhs=wsc_t[:], start=True, stop=True)
                V.tensor_tensor(out=sc_a[:], in0=nm_ps[:, 0:40],
                                in1=an_t[:, ds(b, 1)].to_broadcast([P, 40]),
                                op=OP.mult)
                poly_sincos(V, trig[:, 0:1], trig[:, 1:2], trig[:, 2:3],
                            y_a[:, 64:65])
                # y40 = cos*sc_a + sin*conv_a
                V.tensor_tensor(out=y40[:], in0=sc_a[:],
                                in1=trig[:, 2:3].to_broadcast([P, 40]), op=OP.mult)
                V.tensor_tensor(out=sc_a[:], in0=y_a[:, 0:40],
                                in1=trig[:, 1:2].to_broadcast([P, 40]), op=OP.mult)
                V.tensor_tensor(out=y40[:], in0=y40[:], in1=sc_a[:], op=OP.add)
                # h block
                nc.scalar.activation(sgm[:], y40[:, 0:32], AF.Sigmoid)
                V.tensor_tensor(out=h_blk[:, 0:32], in0=y40[:, 0:32], in1=sgm[:],
                                op=OP.mult)
                nc.scalar.activation(sgm[:, 0:8], y40[:, 32:40], AF.Sigmoid)
                for c in range(3):
                    V.tensor_tensor(out=h_blk[:, 32 + c:56:3],
                                    in0=y_a[:, 40 + c:64:3], in1=sgm[:, 0:8],
                                    op=OP.mult)
                # y0|y1|sc2
                nc.tensor.transpose(out=tp_ps[0:56, :], in_=h_blk[:], identity=ident[:])
                V.tensor_copy(tpc_s[0:56, :], tp_ps[0:56, :])
                nc.tensor.matmul(nm_ps[:, 0:64], lhsT=tpc_s[0:56, :], rhs=w2n_t[:],
                                 start=True, stop=True)
                V.tensor_tensor(out=y01_sb[:], in0=nm_ps[:, 0:64],
                                in1=an_t[:, ds(b, 1)].to_broadcast([P, 64]),
                                op=OP.mult)
                nc.sync.dma_start(out=y01_loc[ds(b * P, P), :], in_=y01_sb[:])

            # ================= AllGather =================
            nc.gpsimd.collective_compute(
                "AllGather",
                OP.bypass,
                replica_groups=[list(range(NC))],
                ins=[y01_loc[:]],
                outs=[y01_all[:]],
            )

            # ================= L2 =================
            with tc.For_i(0, NBLK) as b:
                nc.sync.dma_start(out=sc2_sb[:],
                                  in_=y01_loc[ds(b * P, P), 56:64])
                if debug:
                    nc.sync.dma_start(out=y01_dbg[ds(b * P, P), :],
                                      in_=y01_loc[ds(b * P, P), :])
                for u in range(SUPB):
                    nc.sync.dma_start(
                        out=blob_h[:],
                        in_=blob[:, ds((b * SUPB + u) * 4 * 5, 4 * 5)])
                    V.tensor_copy(blob32[:], blob_h[:])
                    nc.sync.dma_start(
                        out=w2_t2[:],
                        in_=w2scr[ds((b * SUPB + u) * 4 * P, 4 * P), :])
                    nc.sync.dma_start(
                        out=idx_t[:], in_=srcidx[:, ds((b * SUPB + u) * 4, 4)])
                    for j in range(4):
                        nc.gpsimd.indirect_dma_start(
                            out=g_s[:, j, :], out_offset=None,
                            in_=y01_all[:],
                            in_offset=bass.IndirectOffsetOnAxis(
                                ap=idx_t[:, j:j + 1], axis=0))
                    # ef2
                    V.tensor_tensor(out=ef2_s[:, :, 0:32], in0=w2_t2[:, :, 0:32],
                                    in1=g_s[:, :, 0:32], op=OP.mult)
                    V.tensor_tensor(out=ef2_s[:, :, 0:32], in0=ef2_s[:, :, 0:32],
                                    in1=blob32[:, :, 0:1].to_broadcast([P, 4, 32]),
                                    op=OP.mult)
                    V.tensor_tensor(out=tmp8a[:], in0=g_s[:, :, 32:56:3],
                                    in1=blob32[:, :, 1:2].to_broadcast([P, 4, 8]),
                                    op=OP.mult)
                    V.tensor_tensor(out=tmp8b[:], in0=g_s[:, :, 33:56:3],
                                    in1=blob32[:, :, 2:3].to_broadcast([P, 4, 8]),
                                    op=OP.mult)
                    V.tensor_tensor(out=tmp8a[:], in0=tmp8a[:], in1=tmp8b[:],
                                    op=OP.add)
                    V.tensor_tensor(out=tmp8b[:], in0=g_s[:, :, 34:56:3],
                                    in1=blob32[:, :, 3:4].to_broadcast([P, 4, 8]),
                                    op=OP.mult)
                    V.tensor_tensor(out=tmp8a[:], in0=tmp8a[:], in1=tmp8b[:],
                                    op=OP.add)
                    V.tensor_tensor(out=ef2_s[:, :, 32:40], in0=w2_t2[:, :, 32:40],
                                    in1=tmp8a[:], op=OP.mult)
                    for j in range(4):
                        V.tensor_tensor(
                            out=ind_s[:],
                            in0=blob32[:, j, 4:5].to_broadcast([P, P]),
                            in1=iota_f[:], op=OP.is_equal)
                        nc.tensor.matmul(mid2_ps[:], lhsT=ind_s[:],
                                         rhs=ef2_s[:, j, :],
                                         start=(u == 0 and j == 0),
                                         stop=(u == SUPB - 1 and j == 3))
                # ---- node math L2 ----
                nc.scalar.copy(mid2_sb[:], mid2_ps[:])
                if debug:
                    nc.sync.dma_start(out=mid2_dbg[ds(b * P, P), :], in_=mid2_sb[:])
                nc.tensor.transpose(out=tp_ps[0:40, :], in_=mid2_sb[:],
                                    identity=ident[:])
                V.tensor_copy(tpc_s[0:40, :], tp_ps[0:40, :])
                nc.tensor.matmul(nm_ps[:, 0:9], lhsT=tpc_s[0:40, :], rhs=w2b_t[:],
                                 start=True, stop=True)
                V.tensor_tensor(out=y2a[:], in0=nm_ps[:, 0:9],
                                in1=an_t[:, ds(b, 1)].to_broadcast([P, 9]),
                                op=OP.mult)
                poly_sincos(V, trig[:, 0:1], trig[:, 1:2], trig[:, 2:3],
                            y2a[:, 8:9])
                V.tensor_tensor(out=fo[:], in0=sc2_sb[:],
                                in1=trig[:, 2:3].to_broadcast([P, 8]), op=OP.mult)
                V.tensor_tensor(out=y2a[:, 0:8], in0=y2a[:, 0:8],
                                in1=trig[:, 1:2].to_broadcast([P, 8]), op=OP.mult)
                V.tensor_tensor(out=fo[:], in0=fo[:], in1=y2a[:, 0:8], op=OP.add)
                nc.sync.dma_start(out=out_d[ds(b * P, P), :], in_=fo[:])

    nc.compile()
    return nc


def _prep_inputs(node_features, node_attr, edge_attr, edge_scalars,
                 sc1_w, lin1_w, fc1_w1, fc1_w2, lin2_w0, lin2_w1, lin3_w,
                 sc2_w, lin1b_w0, lin1b_w1, fc2_w1, fc2_w2, lin2b_w, lin3b_w,
                 edge_src, edge_dst):
    f = np.float32
    x = np.asarray(node_features, f)
    a = np.asarray(node_attr, f)
    ea = np.asarray(edge_attr, f)
    es = np.asarray(edge_scalars, f)
    src = np.asarray(edge_src).astype(np.int64)
    dst = np.asarray(edge_dst).astype(np.int64)

    inv_nn = 0.25
    S3 = np.sqrt(3.0)
    inv32, inv8, inv40 = 1 / np.sqrt(32.0), 1 / np.sqrt(8.0), 1 / np.sqrt(40.0)

    # ---- weights ----
    w1raw = np.concatenate([np.asarray(fc1_w1, f) / 4.0,
                            np.asarray(fc2_w1, f) / 4.0], axis=1)
    w1cat = np.zeros((17, 128), f)
    w1cat[0:16] = w1raw / 256.0
    w1cat[16] = (0.5 / 256.0) * w1raw.sum(axis=0)
    w1cat = w1cat.astype(np.float16)
    w2bd = np.zeros((128, 72), f)
    w2bd[0:64, 0:32] = np.asarray(fc1_w2, f) * (inv_nn / 8.0)
    w2bd[64:128, 32:72] = np.asarray(fc2_w2, f) * (inv_nn / 8.0)
    w2bd[64:128, 64:72] /= S3
    wnode = np.zeros((64, 65), f)
    wnode[0:16, 0:40] = np.asarray(lin2_w0, f) / 4.0
    wnode[0:16, 64] = 0.1 / 4.0 * np.asarray(lin3_w, f)[:, 0]
    l2w1 = np.asarray(lin2_w1, f) / 4.0
    for u in range(16):
        for w in range(8):
            for c in range(3):
                wnode[16 + 3 * u + c, 40 + 3 * w + c] = l2w1[u, w]
    wsc = (np.asarray(sc1_w, f) / 4.0).astype(f)
    w2n = np.zeros((56, 64), f)
    w2n[0:32, 0:32] = np.asarray(lin1b_w0, f) * inv32
    w2n[0:32, 56:64] = np.asarray(sc2_w, f) * inv32
    l1bw1 = np.asarray(lin1b_w1, f) * inv8
    for u in range(8):
        for w in range(8):
            for c in range(3):
                w2n[32 + 3 * u + c, 32 + 3 * w + c] = l1bw1[u, w]
    w2b = np.zeros((40, 9), f)
    w2b[:, 0:8] = np.asarray(lin2b_w, f) * inv40
    w2b[:, 8] = 0.1 * inv40 * np.asarray(lin3b_w, f)[:, 0]

    # ---- per-node ----
    xf_a = (x @ np.asarray(lin1_w, f)) / 4.0 * a     # [N,16]
    x_pad = np.zeros((N_PAD, 16), f)
    x_pad[:N] = x
    a_pad = np.zeros((N_PAD, 1), f)
    a_pad[:N] = a

    # ---- node rebalancing: deal nodes into degree-balanced 128-node blocks ----
    deg = np.bincount(dst, minlength=N_PAD)
    order = np.argsort(-deg, kind="stable")
    nbins = NC * NBLK
    loads = np.zeros(nbins, np.int64)
    new_of_old = np.empty(N_PAD, np.int64)
    for r in range(P):
        chunk = order[r * nbins:(r + 1) * nbins]
        bin_order = np.argsort(loads, kind="stable")
        new_of_old[chunk] = bin_order * P + r
        loads[bin_order] += deg[chunk]
    assert loads.max() <= SLOTS_BLK, f"block overflow after dealing: {loads.max()}"
    old_of_new = np.argsort(new_of_old, kind="stable")
    dst = new_of_old[dst]
    src = new_of_old[src]
    x_pad = x_pad[old_of_new]
    a_pad = a_pad[old_of_new]

    # ---- edge sort & slotting ----
    perm = np.argsort(dst, kind="stable")
    dst_s = dst[perm]
    src_s = src[perm]
    gb = (dst_s // P).astype(np.int64)              # global block 0..391
    blk_counts = np.bincount(gb, minlength=NC * NBLK)
    assert blk_counts.max() <= SLOTS_BLK, f"block overflow: {blk_counts.max()}"
    starts = np.zeros(NC * NBLK + 1, np.int64)
    np.cumsum(blk_counts, out=starts[1:])
    rank = np.arange(E, dtype=np.int64) - starts[gb]
    slot = gb * SLOTS_BLK + rank                     # global slot id

    S_ALL = NC * SLOTS_CORE
    es_pack = np.zeros((S_ALL, 16), np.uint8)
    es_pack[slot] = np.minimum(np.floor(es[perm] * 256.0), 255.0).astype(np.uint8)
    blob = np.zeros((S_ALL, 5), np.float16)
    blob[:, 4] = -1.0
    blob[slot, 0] = ea[perm, 0].astype(np.float16)
    blob[slot, 1:4] = ea[perm, 1:4].astype(np.float16)
    blob[slot, 4] = (dst_s - gb * P).astype(np.float16)
    srcidx = np.zeros((S_ALL,), np.int32)
    srcidx[slot] = src_s.astype(np.int32)

    xf_pad = np.zeros((N_PAD, 16), np.float16)
    xf_pad[:N] = xf_a.astype(np.float16)
    xf_pad = xf_pad[old_of_new]

    # global [NC*dim0, ...] arrays, matching shard_map's concat layout
    cat = {
        "es_t": np.ascontiguousarray(
            es_pack.reshape(NC, SLOTS_CORE, 16).transpose(0, 2, 1)
        ).reshape(NC * 16, SLOTS_CORE),
        "blob": np.ascontiguousarray(
            blob.reshape(NC, NSUB, P, 5).transpose(0, 2, 1, 3)
        ).reshape(NC * P, NSUB * 5),
        "srcidx": np.ascontiguousarray(
            srcidx.reshape(NC, NSUB, P).transpose(0, 2, 1)
        ).reshape(NC * P, NSUB),
        "xfg": np.tile(xf_pad, (NC, 1)),
        "xnt": np.ascontiguousarray(
            x_pad.reshape(NC, NODES_CORE, 16).transpose(0, 2, 1)
        ).astype(np.float16).reshape(NC * 16, NODES_CORE),
        "an": np.ascontiguousarray(
            a_pad[:, 0].reshape(NC, NBLK, P).transpose(0, 2, 1)
        ).reshape(NC * P, NBLK),
        "w1cat": np.tile(w1cat, (NC, 1)),
        "w2bd": np.tile(w2bd, (NC, 1)),
        "wnode": np.tile(wnode, (NC, 1)),
        "wsc": np.tile(wsc, (NC, 1)),
        "w2n": np.tile(w2n, (NC, 1)),
        "w2b": np.tile(w2b, (NC, 1)),
    }
    return cat, new_of_old


def _get_runner():
    """Build (once) and cache a jitted SPMD callable for the compiled Bass
    module, mirroring bass2jax.run_bass_via_pjrt. Re-using it across calls
    skips the per-call jax retrace + XLA/walrus compile."""
    if "runner" in _CACHED:
        return _CACHED["runner"]
    import jax
    from jax.experimental.shard_map import shard_map
    from jax.sharding import Mesh, PartitionSpec
    from concourse import bass2jax, mybir
    import concourse.bass  # noqa: F401

    bass2jax.install_neuronx_cc_hook()
    nc = _CACHED["nc"]
    assert nc.dbg_addr is None
    partition_name = nc.partition_id_tensor.name if nc.partition_id_tensor else None

    in_names, out_names, out_avals, zero_shapes = [], [], [], []
    for alloc in nc.m.functions[0].allocations:
        if not isinstance(alloc, mybir.MemoryLocationSet):
            continue
        name = alloc.memorylocations[0].name
        if alloc.kind == "ExternalInput":
            if name != partition_name:
                in_names.append(name)
        elif alloc.kind == "ExternalOutput":
            shape = tuple(alloc.tensor_shape)
            dtype = mybir.dt.np(alloc.dtype)
            out_names.append(name)
            out_avals.append(jax.core.ShapedArray(shape, dtype))
            zero_shapes.append((shape, dtype))
    n_params = len(in_names)
    all_names = list(in_names) + list(out_names)
    if partition_name is not None:
        all_names.append(partition_name)
    donate = tuple(range(n_params, n_params + len(out_names)))

    def _body(*args):
        operands = list(args)
        if partition_name is not None:
            operands.append(bass2jax.partition_id_tensor())
        outs = bass2jax._bass_exec_p.bind(
            *operands,
            out_avals=tuple(out_avals),
            in_names=tuple(all_names),
            out_names=tuple(out_names),
            lowering_input_output_aliases=(),
            sim_require_finite=True,
            sim_require_nnan=True,
            nc=nc,
        )
        return tuple(outs)

    devices = jax.devices()[:NC]
    mesh = Mesh(np.asarray(devices), ("core",))
    in_specs = (PartitionSpec("core"),) * (n_params + len(out_names))
    out_specs = (PartitionSpec("core"),) * len(out_names)
    sharded = jax.jit(
        shard_map(_body, mesh=mesh, in_specs=in_specs, out_specs=out_specs,
                  check_rep=False),
        donate_argnums=donate, keep_unused=True)
    _CACHED["runner"] = (sharded, in_names, out_names, zero_shapes)
    return _CACHED["runner"]


def _run(cat):
    sharded, in_names, out_names, zero_shapes = _get_runner()
    concat_in = [cat[nm] for nm in in_names]
    concat_zeros = [
        np.zeros((NC * s[0], *s[1:]), dt) for s, dt in zero_shapes]
    out_arrs = sharded(*concat_in, *concat_zeros)
    return [
        {nm: np.asarray(out_arrs[i]).reshape(NC, *zero_shapes[i][0])[c]
         for i, nm in enumerate(out_names)}
        for c in range(NC)]


def _split_maps(cat, nc):
    """Per-core in_maps view of the global arrays (debug path)."""
    import concourse.mybir as mybir
    dims = {}
    for alloc in nc.m.functions[0].allocations:
        if isinstance(alloc, mybir.MemoryLocationSet) and alloc.kind == "ExternalInput":
            dims[alloc.memorylocations[0].name] = tuple(alloc.tensor_shape)
    return [
        {nm: cat[nm].reshape(NC, *dims[nm])[k] for nm in cat}
        for k in range(NC)]


def _warmup():
    """Pay bass-build + walrus compile + NEFF load + device/comm init once at
    import time with synthetic (structurally valid) inputs, so the real call
    runs the warm path."""
    _CACHED["nc"] = _build_bass(debug=False)
    f = np.float32
    dummy = {
        "node_features": np.zeros((N, 16), f),
        "node_attr": np.ones((N, 1), f),
        "edge_attr": np.zeros((E, 4), f),
        "edge_scalars": np.zeros((E, 16), f),
        "sc1_w": np.zeros((16, 40), f), "lin1_w": np.zeros((16, 16), f),
        "fc1_w1": np.zeros((16, 64), f), "fc1_w2": np.zeros((64, 32), f),
        "lin2_w0": np.zeros((16, 40), f), "lin2_w1": np.zeros((16, 8), f),
        "lin3_w": np.zeros((16, 1), f),
        "sc2_w": np.zeros((32, 8), f), "lin1b_w0": np.zeros((32, 32), f),
        "lin1b_w1": np.zeros((8, 8), f), "fc2_w1": np.zeros((16, 64), f),
        "fc2_w2": np.zeros((64, 40), f), "lin2b_w": np.zeros((40, 8), f),
        "lin3b_w": np.zeros((40, 1), f),
        "edge_src": (np.arange(E, dtype=np.int64) * 7919) % N,
        "edge_dst": np.arange(E, dtype=np.int64) % N,
    }
    cat, _ = _prep_inputs(**dummy)
    _run(cat)


try:
    _warmup()
except Exception:
    _CACHED.pop("runner", None)


def kernel(node_features, node_attr, edge_attr, edge_scalars,
           sc1_w, lin1_w, fc1_w1, fc1_w2, lin2_w0, lin2_w1, lin3_w,
           sc2_w, lin1b_w0, lin1b_w1, fc2_w1, fc2_w2, lin2b_w, lin3b_w,
           edge_src, edge_dst, _debug=False):
    global LAST_EXEC_NS
    from concourse.bass_utils import run_bass_kernel_spmd

    key = "nc_dbg" if _debug else "nc"
    if key not in _CACHED:
        _CACHED[key] = _build_bass(debug=_debug)
    nc = _CACHED[key]

    cat, new_of_old = _prep_inputs(
        node_features, node_attr, edge_attr, edge_scalars,
        sc1_w, lin1_w, fc1_w1, fc1_w2, lin2_w0, lin2_w1, lin3_w,
        sc2_w, lin1b_w0, lin1b_w1, fc2_w1, fc2_w2, lin2b_w, lin3b_w,
        edge_src, edge_dst)
    _CACHED["new_of_old"] = new_of_old

    t0 = time.perf_counter()
    if _debug:
        results = run_bass_kernel_spmd(nc, _split_maps(cat, nc),
                                       list(range(NC))).results
    else:
        try:
            results = _run(cat)
        except Exception:
            results = run_bass_kernel_spmd(nc, _split_maps(cat, nc),
                                           list(range(NC))).results
    t1 = time.perf_counter()
    LAST_EXEC_NS = int((t1 - t0) * 1e9)

    out = np.empty((N_PAD, 8), np.float32)
    for k in range(NC):
        out[k * NODES_CORE:(k + 1) * NODES_CORE] = np.asarray(results[k]["out_d"])
    if _debug:
        dbg = {nm: np.concatenate(
            [np.asarray(results[k][nm]) for k in range(NC)], axis=0)[new_of_old]
            for nm in ("mid_dbg", "y01_dbg", "mid2_dbg")}
        return out[new_of_old[:N]], dbg
    return out[new_of_old[:N]]


# revision 13
# speedup vs baseline: 1.3304x; 1.2667x over previous
"""Trainium2 kernel for nn_MessagePassing_22497038696556 (gnn_message_passing).

Single-NEFF, fully-on-device design (edge-parallel over 8 cores, dst-sharded):
  - Nodes are dealt degree-balanced into 128-node blocks (49 blocks/core,
    50176 total incl. pad; every block carries 2038-2044 edges), so each
    block fits exactly 4 "supertiles" of 512 edge slots (2048 slots, <1%%
    pad). Edges are sorted by the permuted dst; pad slots have dst_rel = -1.
    The host un-permutes the final output rows.
  - Per core, one Bass kernel does EVERYTHING:
      L1: gather xf[src] via indirect DMA from a replicated fp16 table ->
          edge MLP (fp16 matmuls) -> per-edge TP -> segment-sum into the
          node dim via iota/is_equal indicator matmuls accumulating in PSUM
          -> layer-1 node math (transpose + stacked-weight matmuls, Taylor
          sin/cos) -> y0|y1|sc2 table [6272, 64] per core.
      One AllGather (NeuronLink) -> y01 table for all 50176 nodes.
      L2: per-edge gather of y01[src] via indirect DMA -> TP2 -> indicator
          matmul segment-sum -> layer-2 node math -> out [6272, 8].
  - Host only sorts/packs inputs and concatenates the 8 outputs. Edge
    scalars travel as uint8 fixed-point (exact-integer fp16 matmul with a
    ones-row bias fold); other per-edge data as fp16.
  - Import-time warmup with synthetic inputs pays bass-build + walrus
    compile + NEFF load + comm init once; the jitted SPMD callable is
    cached, so kernel() itself is transfers (~39MB in / 1.6MB out through
    the PJRT path) + ~0.1s device exec.
"""

import time
import numpy as np

N = 50000
E = 800000
NC = 8
P = 128
NBLK = 49                      # node blocks per core
NODES_CORE = NBLK * P          # 6272
N_PAD = NC * NODES_CORE        # 50176
SUPB = 4                       # supertiles per block (nodes are re-dealt into degree-balanced blocks)
SLOTS_BLK = SUPB * 512         # 2048
TOTSUP = NBLK * SUPB           # 196 supertiles per core
SLOTS_CORE = NBLK * SLOTS_BLK  # 100352
NSUB = TOTSUP * 4              # 784 subtiles per core

LAST_EXEC_NS = None
_CACHED = {}


def _build_bass(debug=False):
    import concourse.bass as bass
    import concourse.mybir as mybir
    import concourse.tile as tile
    from concourse import bacc
    from concourse.bass import ds
    from concourse.masks import make_identity

    f32 = mybir.dt.float32
    f16 = mybir.dt.float16
    i32 = mybir.dt.int32
    u8 = mybir.dt.uint8
    AF = mybir.ActivationFunctionType
    OP = mybir.AluOpType

    nc = bacc.Bacc(None, target_bir_lowering=False, num_devices=NC)

    # ---- inputs ----
    es_t = nc.dram_tensor("es_t", [16, SLOTS_CORE], u8, kind="ExternalInput")
    blob = nc.dram_tensor("blob", [P, NSUB * 5], f16, kind="ExternalInput")
    xfg = nc.dram_tensor("xfg", [NODES_CORE, 16], f16, kind="ExternalInput")
    srcidx = nc.dram_tensor("srcidx", [P, NSUB], i32, kind="ExternalInput")
    xnt = nc.dram_tensor("xnt", [16, NODES_CORE], f16, kind="ExternalInput")
    an = nc.dram_tensor("an", [P, NBLK], f32, kind="ExternalInput")
    w1cat = nc.dram_tensor("w1cat", [17, 128], f16, kind="ExternalInput")
    w2bd = nc.dram_tensor("w2bd", [128, 72], f32, kind="ExternalInput")
    wnode = nc.dram_tensor("wnode", [64, 65], f32, kind="ExternalInput")
    wsc = nc.dram_tensor("wsc", [16, 40], f32, kind="ExternalInput")
    w2n = nc.dram_tensor("w2n", [56, 64], f32, kind="ExternalInput")
    w2b = nc.dram_tensor("w2b", [40, 9], f32, kind="ExternalInput")
    # ---- outputs ----
    out_d = nc.dram_tensor("out_d", [NODES_CORE, 8], f32, kind="ExternalOutput")
    if debug:
        mid_dbg = nc.dram_tensor("mid_dbg", [NODES_CORE, 64], f32, kind="ExternalOutput")
        y01_dbg = nc.dram_tensor("y01_dbg", [NODES_CORE, 64], f32, kind="ExternalOutput")
        mid2_dbg = nc.dram_tensor("mid2_dbg", [NODES_CORE, 40], f32, kind="ExternalOutput")

    y01_all = nc.dram_tensor("y01_all", [N_PAD, 64], f32, kind="Internal",
                             addr_space="Shared")
    xf_all = nc.dram_tensor("xf_all", [N_PAD, 16], f16, kind="Internal",
                            addr_space="Shared")

    def poly_sincos(v, t2, sin_o, cos_o, ang_ap):
        # t2 = ang^2; sin = ang*(1 + t2*(-1/6 + t2*(1/120 - t2/5040)))
        # cos = 1 + t2*(-1/2 + t2*(1/24 + t2*(-1/720 + t2/40320)))
        v.tensor_tensor(out=t2[:], in0=ang_ap, in1=ang_ap, op=OP.mult)
        v.tensor_scalar(out=sin_o[:], in0=t2[:], scalar1=-1.0 / 5040.0,
                        scalar2=1.0 / 120.0, op0=OP.mult, op1=OP.add)
        v.tensor_tensor(out=sin_o[:], in0=t2[:], in1=sin_o[:], op=OP.mult)
        v.tensor_scalar(out=sin_o[:], in0=sin_o[:], scalar1=1.0,
                        scalar2=-1.0 / 6.0, op0=OP.mult, op1=OP.add)
        v.tensor_tensor(out=sin_o[:], in0=t2[:], in1=sin_o[:], op=OP.mult)
        v.tensor_scalar(out=sin_o[:], in0=sin_o[:], scalar1=1.0,
                        scalar2=1.0, op0=OP.mult, op1=OP.add)
        v.tensor_tensor(out=sin_o[:], in0=ang_ap, in1=sin_o[:], op=OP.mult)
        v.tensor_scalar(out=cos_o[:], in0=t2[:], scalar1=1.0 / 40320.0,
                        scalar2=-1.0 / 720.0, op0=OP.mult, op1=OP.add)
        v.tensor_tensor(out=cos_o[:], in0=t2[:], in1=cos_o[:], op=OP.mult)
        v.tensor_scalar(out=cos_o[:], in0=cos_o[:], scalar1=1.0,
                        scalar2=1.0 / 24.0, op0=OP.mult, op1=OP.add)
        v.tensor_tensor(out=cos_o[:], in0=t2[:], in1=cos_o[:], op=OP.mult)
        v.tensor_scalar(out=cos_o[:], in0=cos_o[:], scalar1=1.0,
                        scalar2=-0.5, op0=OP.mult, op1=OP.add)
        v.tensor_tensor(out=cos_o[:], in0=t2[:], in1=cos_o[:], op=OP.mult)
        v.tensor_scalar(out=cos_o[:], in0=cos_o[:], scalar1=1.0,
                        scalar2=1.0, op0=OP.mult, op1=OP.add)

    with tile.TileContext(nc) as tc:
        with (
            tc.tile_pool(name="const", bufs=1) as cp,
            tc.tile_pool(name="sb", bufs=1) as sb,
            tc.tile_pool(name="ps", bufs=1, space="PSUM") as psp,
            tc.tile_pool(name="dram", bufs=1, space="DRAM") as dp,
        ):
            V = nc.vector

            # ------- constants / weights -------
            iota_i = cp.tile([P, P], i32, tag="iota_i")
            nc.gpsimd.iota(iota_i[:], pattern=[[1, P]], base=0, channel_multiplier=0)
            iota_f = cp.tile([P, P], f32, tag="iota_f")
            V.tensor_copy(iota_f[:], iota_i[:])
            ident = cp.tile([P, P], f32, tag="ident")
            make_identity(nc, ident[:])

            w1_t = cp.tile([17, 128], f16, tag="w1")
            nc.sync.dma_start(out=w1_t[:], in_=w1cat[:])
            w2bd_t = cp.tile([128, 72], f32, tag="w2bd")
            nc.sync.dma_start(out=w2bd_t[:], in_=w2bd[:])
            wnode_t = cp.tile([64, 65], f32, tag="wnode")
            nc.sync.dma_start(out=wnode_t[:], in_=wnode[:])
            wsc_t = cp.tile([16, 40], f32, tag="wsc")
            nc.sync.dma_start(out=wsc_t[:], in_=wsc[:])
            w2n_t = cp.tile([56, 64], f32, tag="w2n")
            nc.sync.dma_start(out=w2n_t[:], in_=w2n[:])
            w2b_t = cp.tile([40, 9], f32, tag="w2b")
            nc.sync.dma_start(out=w2b_t[:], in_=w2b[:])

            xn_h = sb.tile([16, P], f16, tag="xn_h")
            xn32 = sb.tile([16, P], f32, tag="xn32")
            an_t = cp.tile([P, NBLK], f32, tag="an")
            nc.sync.dma_start(out=an_t[:], in_=an[:])

            # ------- static SBUF workspaces -------
            es_u8 = sb.tile([16, 512], u8, tag="es_u8")
            es_s = cp.tile([17, 512], f16, tag="es_s")
            V.memset(es_s[:], 1.0)
            blob_h = sb.tile([P, 4, 5], f16, tag="blob_h")
            blob32 = sb.tile([P, 4, 5], f32, tag="blob32")
            xs_h = sb.tile([P, 4, 16], f16, tag="xs_h")
            xs32 = sb.tile([P, 4, 16], f32, tag="xs32")
            sg_s = sb.tile([P, 512], f32, tag="sg")
            h_s = sb.tile([P, 512], f32, tag="h")
            w_sb = sb.tile([P, 4, 72], f32, tag="w_sb")
            tmp16a = sb.tile([P, 4, 16], f32, tag="tmp16a")
            tmp16b = sb.tile([P, 4, 16], f32, tag="tmp16b")
            ef_s = sb.tile([P, 4, 64], f32, tag="ef")
            ind_s = sb.tile([P, P], f32, tag="ind")
            mid_sb = sb.tile([P, 64], f32, tag="mid_sb")
            tpc_s = sb.tile([P, P], f32, tag="tpc")      # copies of transposes
            y_a = sb.tile([P, 65], f32, tag="y_a")
            sc_a = sb.tile([P, 40], f32, tag="sc_a")
            trig = sb.tile([P, 4], f32, tag="trig")      # t2, sin, cos
            y40 = sb.tile([P, 40], f32, tag="y40")
            sgm = sb.tile([P, 32], f32, tag="sgm")
            h_blk = sb.tile([P, 56], f32, tag="h_blk")
            y01_sb = sb.tile([P, 64], f32, tag="y01_sb")
            # L2 workspaces
            w2_t2 = sb.tile([P, 4, 40], f32, tag="w2_t2")
            idx_t = sb.tile([P, 4], i32, tag="idx_t")
            g_s = sb.tile([P, 4, 64], f32, tag="g_s")
            ef2_s = sb.tile([P, 4, 40], f32, tag="ef2")
            tmp8a = sb.tile([P, 4, 8], f32, tag="tmp8a")
            tmp8b = sb.tile([P, 4, 8], f32, tag="tmp8b")
            mid2_sb = sb.tile([P, 40], f32, tag="mid2_sb")
            sc2_sb = sb.tile([P, 8], f32, tag="sc2_sb")
            y2a = sb.tile([P, 9], f32, tag="y2a")
            fo = sb.tile([P, 8], f32, tag="fo")

            # ------- PSUM tiles (8 banks) -------
            h_ps = psp.tile([P, 512], f32, tag="h_ps")
            w_ps = psp.tile([P, 72], f32, tag="w_ps")
            mid_ps = psp.tile([P, 64], f32, tag="mid_ps")
            tp_ps = psp.tile([P, P], f32, tag="tp_ps")
            nm_ps = psp.tile([P, 128], f32, tag="nm_ps")
            mid2_ps = psp.tile([P, 40], f32, tag="mid2_ps")

            # ------- DRAM scratch -------
            w2scr = dp.tile([NSUB * P, 40], f32, tag="w2scr")
            y01_loc = dp.tile([NODES_CORE, 64], f32, tag="y01_loc")
            xf_bounce = dp.tile([NODES_CORE, 16], f16, tag="xf_bounce")

            # ---- assemble the full xf gather table on device ----
            nc.gpsimd.dma_start(out=xf_bounce[:], in_=xfg[:])
            nc.gpsimd.collective_compute(
                "AllGather",
                OP.bypass,
                replica_groups=[list(range(NC))],
                ins=[xf_bounce[:]],
                outs=[xf_all[:]],
            )

            # ================= L1 =================
            with tc.For_i(0, NBLK) as b:
                for u in range(SUPB):
                    nc.sync.dma_start(
                        out=es_u8[:], in_=es_t[:, ds(b * SLOTS_BLK + u * 512, 512)])
                    V.tensor_copy(es_s[0:16, :], es_u8[:])
                    nc.sync.dma_start(
                        out=blob_h[:],
                        in_=blob[:, ds((b * SUPB + u) * 4 * 5, 4 * 5)])
                    V.tensor_copy(blob32[:], blob_h[:])
                    nc.sync.dma_start(
                        out=idx_t[:], in_=srcidx[:, ds((b * SUPB + u) * 4, 4)])
                    for j in range(4):
                        nc.gpsimd.indirect_dma_start(
                            out=xs_h[:, j, :], out_offset=None,
                            in_=xf_all[:],
                            in_offset=bass.IndirectOffsetOnAxis(
                                ap=idx_t[:, j:j + 1], axis=0))
                    V.tensor_copy(xs32[:], xs_h[:])
                    # MLP stage 1: h = silu(es^T @ w1cat)  -> [128 hf, 512 e]
                    nc.tensor.matmul(h_ps[:], lhsT=w1_t[:], rhs=es_s[:],
                                     start=True, stop=True)
                    nc.scalar.activation(sg_s[:], h_ps[:], AF.Sigmoid)
                    V.tensor_tensor(out=h_s[:], in0=h_ps[:], in1=sg_s[:], op=OP.mult)
                    # MLP stage 2 per subtile: w|w2 [128 e, 72]
                    for j in range(4):
                        nc.tensor.matmul(w_ps[:], lhsT=h_s[:, j * 128:(j + 1) * 128],
                                         rhs=w2bd_t[:], start=True, stop=True)
                        V.tensor_copy(w_sb[:, j, :], w_ps[:])
                    # stash w2 for L2
                    nc.sync.dma_start(
                        out=w2scr[ds((b * SUPB + u) * 4 * P, 4 * P), :],
                        in_=w_sb[:, :, 32:72])
                    # TP: ef [128 e, 64]
                    V.tensor_tensor(out=tmp16a[:], in0=w_sb[:, :, 0:16],
                                    in1=xs32[:], op=OP.mult)
                    V.tensor_tensor(out=ef_s[:, :, 0:16], in0=tmp16a[:],
                                    in1=blob32[:, :, 0:1].to_broadcast([P, 4, 16]),
                                    op=OP.mult)
                    V.tensor_tensor(out=tmp16b[:], in0=w_sb[:, :, 16:32],
                                    in1=xs32[:], op=OP.mult)
                    for c in range(3):
                        V.tensor_tensor(
                            out=ef_s[:, :, 16 + c:64:3], in0=tmp16b[:],
                            in1=blob32[:, :, 1 + c:2 + c].to_broadcast([P, 4, 16]),
                            op=OP.mult)
                    # scatter via indicator matmuls
                    for j in range(4):
                        V.tensor_tensor(
                            out=ind_s[:],
                            in0=blob32[:, j, 4:5].to_broadcast([P, P]),
                            in1=iota_f[:], op=OP.is_equal)
                        nc.tensor.matmul(mid_ps[:], lhsT=ind_s[:], rhs=ef_s[:, j, :],
                                         start=(u == 0 and j == 0),
                                         stop=(u == SUPB - 1 and j == 3))
                # ---- node math L1 ----
                nc.scalar.copy(mid_sb[:], mid_ps[:])
                if debug:
                    nc.sync.dma_start(out=mid_dbg[ds(b * P, P), :], in_=mid_sb[:])
                nc.tensor.transpose(out=tp_ps[0:64, :], in_=mid_sb[:], identity=ident[:])
                V.tensor_copy(tpc_s[0:64, :], tp_ps[0:64, :])
                nc.tensor.matmul(nm_ps[:, 0:65], lhsT=tpc_s[0:64, :], rhs=wnode_t[:],
                                 start=True, stop=True)
                V.tensor_tensor(out=y_a[:],
                                in0=nm_ps[:, 0:65],
                                in1=an_t[:, ds(b, 1)].to_broadcast([P, 65]),
                                op=OP.mult)
                nc.sync.dma_start(out=xn_h[:], in_=xnt[:, ds(b * P, P)])
                V.tensor_copy(xn32[:], xn_h[:])
                nc.tensor.matmul(nm_ps[:, 0:40], lhsT=xn32[:],
                                 rhs=wsc_t[:], start=True, stop=True)
                V.tensor_tensor(out=sc_a[:], in0=nm_ps[:, 0:40],
                                in1=an_t[:, ds(b, 1)].to_broadcast([P, 40]),
                                op=OP.mult)
                poly_sincos(V, trig[:, 0:1], trig[:, 1:2], trig[:, 2:3],
                            y_a[:, 64:65])
                # y40 = cos*sc_a + sin*conv_a
                V.tensor_tensor(out=y40[:], in0=sc_a[:],
                                in1=trig[:, 2:3].to_broadcast([P, 40]), op=OP.mult)
                V.tensor_tensor(out=sc_a[:], in0=y_a[:, 0:40],
                                in1=trig[:, 1:2].to_broadcast([P, 40]), op=OP.mult)
                V.tensor_tensor(out=y40[:], in0=y40[:], in1=sc_a[:], op=OP.add)
                # h block
                nc.scalar.activation(sgm[:], y40[:, 0:32], AF.Sigmoid)
                V.tensor_tensor(out=h_blk[:, 0:32], in0=y40[:, 0:32], in1=sgm[:],
                                op=OP.mult)
                nc.scalar.activation(sgm[:, 0:8], y40[:, 32:40], AF.Sigmoid)
                for c in range(3):
                    V.tensor_tensor(out=h_blk[:, 32 + c:56:3],
                                    in0=y_a[:, 40 + c:64:3], in1=sgm[:, 0:8],
                                    op=OP.mult)
                # y0|y1|sc2
                nc.tensor.transpose(out=tp_ps[0:56, :], in_=h_blk[:], identity=ident[:])
                V.tensor_copy(tpc_s[0:56, :], tp_ps[0:56, :])
                nc.tensor.matmul(nm_ps[:, 0:64], lhsT=tpc_s[0:56, :], rhs=w2n_t[:],
                                 start=True, stop=True)
                V.tensor_tensor(out=y01_sb[:], in0=nm_ps[:, 0:64],
                                in1=an_t[:, ds(b, 1)].to_broadcast([P, 64]),
                                op=OP.mult)
                nc.sync.dma_start(out=y01_loc[ds(b * P, P), :], in_=y01_sb[:])

            # ================= AllGather =================
            nc.gpsimd.collective_compute(
                "AllGather",
                OP.bypass,
                replica_groups=[list(range(NC))],
                ins=[y01_loc[:]],
                outs=[y01_all[:]],
            )

            # ================= L2 =================
            with tc.For_i(0, NBLK) as b:
                nc.sync.dma_start(out=sc2_sb[:],
                                  in_=y01_loc[ds(b * P, P), 56:64])
                if debug:
                    nc.sync.dma_start(out=y01_dbg[ds(b * P, P), :],
                                      in_=y01_loc[ds(b * P, P), :])
                for u in range(SUPB):
                    nc.sync.dma_start(
                        out=blob_h[:],
                        in_=blob[:, ds((b * SUPB + u) * 4 * 5, 4 * 5)])
                    V.tensor_copy(blob32[:], blob_h[:])
                    nc.sync.dma_start(
                        out=w2_t2[:],
                        in_=w2scr[ds((b * SUPB + u) * 4 * P, 4 * P), :])
                    nc.sync.dma_start(
                        out=idx_t[:], in_=srcidx[:, ds((b * SUPB + u) * 4, 4)])
                    for j in range(4):
                        nc.gpsimd.indirect_dma_start(
                            out=g_s[:, j, :], out_offset=None,
                            in_=y01_all[:],
                            in_offset=bass.IndirectOffsetOnAxis(
                                ap=idx_t[:, j:j + 1], axis=0))
                    # ef2
                    V.tensor_tensor(out=ef2_s[:, :, 0:32], in0=w2_t2[:, :, 0:32],
                                    in1=g_s[:, :, 0:32], op=OP.mult)
                    V.tensor_tensor(out=ef2_s[:, :, 0:32], in0=ef2_s[:, :, 0:32],
                                    in1=blob32[:, :, 0:1].to_broadcast([P, 4, 32]),
                                    op=OP.mult)
                    V.tensor_tensor(out=tmp8a[:], in0=g_s[:, :, 32:56:3],
                                    in1=blob32[:, :, 1:2].to_broadcast([P, 4, 8]),
                                    op=OP.mult)
                    V.tensor_tensor(out=tmp8b[:], in0=g_s[:, :, 33:56:3],
                                    in1=blob32[:, :, 2:3].to_broadcast([P, 4, 8]),
                                    op=OP.mult)
                    V.tensor_tensor(out=tmp8a[:], in0=tmp8a[:], in1=tmp8b[:],
                                    op=OP.add)
                    V.tensor_tensor(out=tmp8b[:], in0=g_s[:, :, 34:56:3],
                                    in1=blob32[:, :, 3:4].to_broadcast([P, 4, 8]),
                                    op=OP.mult)
                    V.tensor_tensor(out=tmp8a[:], in0=tmp8a[:], in1=tmp8b[:],
                                    op=OP.add)
                    V.tensor_tensor(out=ef2_s[:, :, 32:40], in0=w2_t2[:, :, 32:40],
                                    in1=tmp8a[:], op=OP.mult)
                    for j in range(4):
                        V.tensor_tensor(
                            out=ind_s[:],
                            in0=blob32[:, j, 4:5].to_broadcast([P, P]),
                            in1=iota_f[:], op=OP.is_equal)
                        nc.tensor.matmul(mid2_ps[:], lhsT=ind_s[:],
                                         rhs=ef2_s[:, j, :],
                                         start=(u == 0 and j == 0),
                                         stop=(u == SUPB - 1 and j == 3))
                # ---- node math L2 ----
                nc.scalar.copy(mid2_sb[:], mid2_ps[:])
                if debug:
                    nc.sync.dma_start(out=mid2_dbg[ds(b * P, P), :], in_=mid2_sb[:])
                nc.tensor.transpose(out=tp_ps[0:40, :], in_=mid2_sb[:],
                                    identity=ident[:])
                V.tensor_copy(tpc_s[0:40, :], tp_ps[0:40, :])
                nc.tensor.matmul(nm_ps[:, 0:9], lhsT=tpc_s[0:40, :], rhs=w2b_t[:],
                                 start=True, stop=True)
                V.tensor_tensor(out=y2a[:], in0=nm_ps[:, 0:9],
                                in1=an_t[:, ds(b, 1)].to_broadcast([P, 9]),
                                op=OP.mult)
                poly_sincos(V, trig[:, 0:1], trig[:, 1:2], trig[:, 2:3],
                            y2a[:, 8:9])
                V.tensor_tensor(out=fo[:], in0=sc2_sb[:],
                                in1=trig[:, 2:3].to_broadcast([P, 8]), op=OP.mult)
                V.tensor_tensor(out=y2a[:, 0:8], in0=y2a[:, 0:8],
                                in1=trig[:, 1:2].to_broadcast([P, 8]), op=OP.mult)
                V.tensor_tensor(out=fo[:], in0=fo[:], in1=y2a[:, 0:8], op=OP.add)
                nc.sync.dma_start(out=out_d[ds(b * P, P), :], in_=fo[:])

    nc.compile()
    return nc


def _prep_inputs(node_features, node_attr, edge_attr, edge_scalars,
                 sc1_w, lin1_w, fc1_w1, fc1_w2, lin2_w0, lin2_w1, lin3_w,
                 sc2_w, lin1b_w0, lin1b_w1, fc2_w1, fc2_w2, lin2b_w, lin3b_w,
                 edge_src, edge_dst):
    f = np.float32
    x = np.asarray(node_features, f)
    a = np.asarray(node_attr, f)
    ea = np.asarray(edge_attr, f)
    es = np.asarray(edge_scalars, f)
    src = np.asarray(edge_src).astype(np.int64)
    dst = np.asarray(edge_dst).astype(np.int64)

    inv_nn = 0.25
    S3 = np.sqrt(3.0)
    inv32, inv8, inv40 = 1 / np.sqrt(32.0), 1 / np.sqrt(8.0), 1 / np.sqrt(40.0)

    # ---- weights ----
    w1raw = np.concatenate([np.asarray(fc1_w1, f) / 4.0,
                            np.asarray(fc2_w1, f) / 4.0], axis=1)
    w1cat = np.zeros((17, 128), f)
    w1cat[0:16] = w1raw / 256.0
    w1cat[16] = (0.5 / 256.0) * w1raw.sum(axis=0)
    w1cat = w1cat.astype(np.float16)
    w2bd = np.zeros((128, 72), f)
    w2bd[0:64, 0:32] = np.asarray(fc1_w2, f) * (inv_nn / 8.0)
    w2bd[64:128, 32:72] = np.asarray(fc2_w2, f) * (inv_nn / 8.0)
    w2bd[64:128, 64:72] /= S3
    wnode = np.zeros((64, 65), f)
    wnode[0:16, 0:40] = np.asarray(lin2_w0, f) / 4.0
    wnode[0:16, 64] = 0.1 / 4.0 * np.asarray(lin3_w, f)[:, 0]
    l2w1 = np.asarray(lin2_w1, f) / 4.0
    for u in range(16):
        for w in range(8):
            for c in range(3):
                wnode[16 + 3 * u + c, 40 + 3 * w + c] = l2w1[u, w]
    wsc = (np.asarray(sc1_w, f) / 4.0).astype(f)
    w2n = np.zeros((56, 64), f)
    w2n[0:32, 0:32] = np.asarray(lin1b_w0, f) * inv32
    w2n[0:32, 56:64] = np.asarray(sc2_w, f) * inv32
    l1bw1 = np.asarray(lin1b_w1, f) * inv8
    for u in range(8):
        for w in range(8):
            for c in range(3):
                w2n[32 + 3 * u + c, 32 + 3 * w + c] = l1bw1[u, w]
    w2b = np.zeros((40, 9), f)
    w2b[:, 0:8] = np.asarray(lin2b_w, f) * inv40
    w2b[:, 8] = 0.1 * inv40 * np.asarray(lin3b_w, f)[:, 0]

    # ---- per-node ----
    xf_a = (x @ np.asarray(lin1_w, f)) / 4.0 * a     # [N,16]
    x_pad = np.zeros((N_PAD, 16), f)
    x_pad[:N] = x
    a_pad = np.zeros((N_PAD, 1), f)
    a_pad[:N] = a

    # ---- node rebalancing: deal nodes into degree-balanced 128-node blocks ----
    deg = np.bincount(dst, minlength=N_PAD)
    order = np.argsort(-deg, kind="stable")
    nbins = NC * NBLK
    loads = np.zeros(nbins, np.int64)
    new_of_old = np.empty(N_PAD, np.int64)
    for r in range(P):
        chunk = order[r * nbins:(r + 1) * nbins]
        bin_order = np.argsort(loads, kind="stable")
        new_of_old[chunk] = bin_order * P + r
        loads[bin_order] += deg[chunk]
    assert loads.max() <= SLOTS_BLK, f"block overflow after dealing: {loads.max()}"
    old_of_new = np.argsort(new_of_old, kind="stable")
    dst = new_of_old[dst]
    src = new_of_old[src]
    x_pad = x_pad[old_of_new]
    a_pad = a_pad[old_of_new]

    # ---- edge sort & slotting ----
    perm = np.argsort(dst, kind="stable")
    dst_s = dst[perm]
    src_s = src[perm]
    gb = (dst_s // P).astype(np.int64)              # global block 0..391
    blk_counts = np.bincount(gb, minlength=NC * NBLK)
    assert blk_counts.max() <= SLOTS_BLK, f"block overflow: {blk_counts.max()}"
    starts = np.zeros(NC * NBLK + 1, np.int64)
    np.cumsum(blk_counts, out=starts[1:])
    rank = np.arange(E, dtype=np.int64) - starts[gb]
    slot = gb * SLOTS_BLK + rank                     # global slot id

    S_ALL = NC * SLOTS_CORE
    es_pack = np.zeros((S_ALL, 16), np.uint8)
    es_pack[slot] = np.minimum(np.floor(es[perm] * 256.0), 255.0).astype(np.uint8)
    blob = np.zeros((S_ALL, 5), np.float16)
    blob[:, 4] = -1.0
    blob[slot, 0] = ea[perm, 0].astype(np.float16)
    blob[slot, 1:4] = ea[perm, 1:4].astype(np.float16)
    blob[slot, 4] = (dst_s - gb * P).astype(np.float16)
    srcidx = np.zeros((S_ALL,), np.int32)
    srcidx[slot] = src_s.astype(np.int32)

    xf_pad = np.zeros((N_PAD, 16), np.float16)
    xf_pad[:N] = xf_a.astype(np.float16)
    xf_pad = xf_pad[old_of_new]

    # global [NC*dim0, ...] arrays, matching shard_map's concat layout
    cat = {
        "es_t": np.ascontiguousarray(
            es_pack.reshape(NC, SLOTS_CORE, 16).transpose(0, 2, 1)
        ).reshape(NC * 16, SLOTS_CORE),
        "blob": np.ascontiguousarray(
            blob.reshape(NC, NSUB, P, 5).transpose(0, 2, 1, 3)
        ).reshape(NC * P, NSUB * 5),
        "srcidx": np.ascontiguousarray(
            srcidx.reshape(NC, NSUB, P).transpose(0, 2, 1)
        ).reshape(NC * P, NSUB),
        "xfg": xf_pad,
        "xnt": np.ascontiguousarray(
            x_pad.reshape(NC, NODES_CORE, 16).transpose(0, 2, 1)
        ).astype(np.float16).reshape(NC * 16, NODES_CORE),
        "an": np.ascontiguousarray(
            a_pad[:, 0].reshape(NC, NBLK, P).transpose(0, 2, 1)
        ).reshape(NC * P, NBLK),
        "w1cat": np.tile(w1cat, (NC, 1)),
        "w2bd": np.tile(w2bd, (NC, 1)),
        "wnode": np.tile(wnode, (NC, 1)),
        "wsc": np.tile(wsc, (NC, 1)),
        "w2n": np.tile(w2n, (NC, 1)),
        "w2b": np.tile(w2b, (NC, 1)),
    }
    return cat, new_of_old


def _get_runner():
    """Build (once) and cache a jitted SPMD callable for the compiled Bass
    module, mirroring bass2jax.run_bass_via_pjrt. Re-using it across calls
    skips the per-call jax retrace + XLA/walrus compile."""
    if "runner" in _CACHED:
        return _CACHED["runner"]
    import jax
    from jax.experimental.shard_map import shard_map
    from jax.sharding import Mesh, PartitionSpec
    from concourse import bass2jax, mybir
    import concourse.bass  # noqa: F401

    bass2jax.install_neuronx_cc_hook()
    nc = _CACHED["nc"]
    assert nc.dbg_addr is None
    partition_name = nc.partition_id_tensor.name if nc.partition_id_tensor else None

    in_names, out_names, out_avals, zero_shapes = [], [], [], []
    for alloc in nc.m.functions[0].allocations:
        if not isinstance(alloc, mybir.MemoryLocationSet):
            continue
        name = alloc.memorylocations[0].name
        if alloc.kind == "ExternalInput":
            if name != partition_name:
                in_names.append(name)
        elif alloc.kind == "ExternalOutput":
            shape = tuple(alloc.tensor_shape)
            dtype = mybir.dt.np(alloc.dtype)
            out_names.append(name)
            out_avals.append(jax.core.ShapedArray(shape, dtype))
            zero_shapes.append((shape, dtype))
    n_params = len(in_names)
    all_names = list(in_names) + list(out_names)
    if partition_name is not None:
        all_names.append(partition_name)
    donate = tuple(range(n_params, n_params + len(out_names)))

    def _body(*args):
        operands = list(args)
        if partition_name is not None:
            operands.append(bass2jax.partition_id_tensor())
        outs = bass2jax._bass_exec_p.bind(
            *operands,
            out_avals=tuple(out_avals),
            in_names=tuple(all_names),
            out_names=tuple(out_names),
            lowering_input_output_aliases=(),
            sim_require_finite=True,
            sim_require_nnan=True,
            nc=nc,
        )
        return tuple(outs)

    devices = jax.devices()[:NC]
    mesh = Mesh(np.asarray(devices), ("core",))
    in_specs = (PartitionSpec("core"),) * (n_params + len(out_names))
    out_specs = (PartitionSpec("core"),) * len(out_names)
    sharded = jax.jit(
        shard_map(_body, mesh=mesh, in_specs=in_specs, out_specs=out_specs,
                  check_rep=False),
        donate_argnums=donate, keep_unused=True)
    _CACHED["runner"] = (sharded, in_names, out_names, zero_shapes)
    return _CACHED["runner"]


def _run(cat):
    sharded, in_names, out_names, zero_shapes = _get_runner()
    concat_in = [cat[nm] for nm in in_names]
    concat_zeros = [
        np.zeros((NC * s[0], *s[1:]), dt) for s, dt in zero_shapes]
    out_arrs = sharded(*concat_in, *concat_zeros)
    return [
        {nm: np.asarray(out_arrs[i]).reshape(NC, *zero_shapes[i][0])[c]
         for i, nm in enumerate(out_names)}
        for c in range(NC)]


def _split_maps(cat, nc):
    """Per-core in_maps view of the global arrays (debug path)."""
    import concourse.mybir as mybir
    dims = {}
    for alloc in nc.m.functions[0].allocations:
        if isinstance(alloc, mybir.MemoryLocationSet) and alloc.kind == "ExternalInput":
            dims[alloc.memorylocations[0].name] = tuple(alloc.tensor_shape)
    return [
        {nm: cat[nm].reshape(NC, *dims[nm])[k] for nm in cat}
        for k in range(NC)]


def _warmup():
    """Pay bass-build + walrus compile + NEFF load + device/comm init once at
    import time with synthetic (structurally valid) inputs, so the real call
    runs the warm path."""
    _CACHED["nc"] = _build_bass(debug=False)
    f = np.float32
    dummy = {
        "node_features": np.zeros((N, 16), f),
        "node_attr": np.ones((N, 1), f),
        "edge_attr": np.zeros((E, 4), f),
        "edge_scalars": np.zeros((E, 16), f),
        "sc1_w": np.zeros((16, 40), f), "lin1_w": np.zeros((16, 16), f),
        "fc1_w1": np.zeros((16, 64), f), "fc1_w2": np.zeros((64, 32), f),
        "lin2_w0": np.zeros((16, 40), f), "lin2_w1": np.zeros((16, 8), f),
        "lin3_w": np.zeros((16, 1), f),
        "sc2_w": np.zeros((32, 8), f), "lin1b_w0": np.zeros((32, 32), f),
        "lin1b_w1": np.zeros((8, 8), f), "fc2_w1": np.zeros((16, 64), f),
        "fc2_w2": np.zeros((64, 40), f), "lin2b_w": np.zeros((40, 8), f),
        "lin3b_w": np.zeros((40, 1), f),
        "edge_src": (np.arange(E, dtype=np.int64) * 7919) % N,
        "edge_dst": np.arange(E, dtype=np.int64) % N,
    }
    cat, _ = _prep_inputs(**dummy)
    _run(cat)


try:
    _warmup()
except Exception:
    _CACHED.pop("runner", None)


def kernel(node_features, node_attr, edge_attr, edge_scalars,
           sc1_w, lin1_w, fc1_w1, fc1_w2, lin2_w0, lin2_w1, lin3_w,
           sc2_w, lin1b_w0, lin1b_w1, fc2_w1, fc2_w2, lin2b_w, lin3b_w,
           edge_src, edge_dst, _debug=False):
    global LAST_EXEC_NS
    from concourse.bass_utils import run_bass_kernel_spmd

    key = "nc_dbg" if _debug else "nc"
    if key not in _CACHED:
        _CACHED[key] = _build_bass(debug=_debug)
    nc = _CACHED[key]

    cat, new_of_old = _prep_inputs(
        node_features, node_attr, edge_attr, edge_scalars,
        sc1_w, lin1_w, fc1_w1, fc1_w2, lin2_w0, lin2_w1, lin3_w,
        sc2_w, lin1b_w0, lin1b_w1, fc2_w1, fc2_w2, lin2b_w, lin3b_w,
        edge_src, edge_dst)
    _CACHED["new_of_old"] = new_of_old

    t0 = time.perf_counter()
    if _debug:
        results = run_bass_kernel_spmd(nc, _split_maps(cat, nc),
                                       list(range(NC))).results
    else:
        try:
            results = _run(cat)
        except Exception:
            results = run_bass_kernel_spmd(nc, _split_maps(cat, nc),
                                           list(range(NC))).results
    t1 = time.perf_counter()
    LAST_EXEC_NS = int((t1 - t0) * 1e9)

    out = np.empty((N_PAD, 8), np.float32)
    for k in range(NC):
        out[k * NODES_CORE:(k + 1) * NODES_CORE] = np.asarray(results[k]["out_d"])
    if _debug:
        dbg = {nm: np.concatenate(
            [np.asarray(results[k][nm]) for k in range(NC)], axis=0)[new_of_old]
            for nm in ("mid_dbg", "y01_dbg", "mid2_dbg")}
        return out[new_of_old[:N]], dbg
    return out[new_of_old[:N]]
